# revision 6
# baseline (speedup 1.0000x reference)
"""Distributed Trainium2 kernel for the ADAGAD GNN message-passing model.

Model (see problem reference): three 2-layer GCN encoders over a shared
graph, attention-softmax fusion of the three embeddings, two GCN decoder
heads, and a final dense similarity matrix s_ = h_ @ h_.T.

Every GCN conv uses the same symmetric-normalized adjacency with self
loops, A_hat = D^-1/2 (A + I) D^-1/2 (D = 1 + in-degree).  The host
pre-bakes A_hat^T densely in bf16 and column-shards it over the 8 cores
(columns = destination nodes, matching the "partition edges by dst"
sharding).  Each core keeps its 16 MB shard resident in SBUF and runs all
sparse aggregations as dense TensorE matmuls in "outT" form:

    aggT[f, m] = sum_k H[k, f] * A_hatT[k, m]   (lhsT = H chunk, rhs = A_hatT)

which produces feature-major aggregates whose slices feed directly as
lhsT into the small dense-weight matmuls, flipping back to node-major
with no transposes anywhere.  Cross-core exchange is three bf16
AllGathers (H1 after encoder layer 1, h after fusion, h_^T before the
final row-sharded h_ @ h_.T whose 32 MB/core f32 output write is the
memory-roofline term).
"""

import numpy as np

N = 8192
IN = 64
HID = 64
F3 = 3 * HID          # 192
NCORES = 8
NB = N // NCORES      # 1024 rows (dst nodes) per core
P = 128               # partitions
KC = N // P           # 64 contraction chunks
MT = NB // P          # 8 m-tiles per core

TRACE = False         # set by test harness to collect HW exec time
LAST_EXEC_NS = None

_PROG = None


def _build_program():
    import concourse.bass as bass
    import concourse.mybir as mybir
    from concourse import bacc
    from concourse.bass import ds
    from concourse.tile import TileContext

    bf16 = mybir.dt.bfloat16
    f32 = mybir.dt.float32
    Relu = mybir.ActivationFunctionType.Relu
    Exp = mybir.ActivationFunctionType.Exp
    Copy = mybir.ActivationFunctionType.Copy
    RG = [list(range(NCORES))]

    nc = bacc.Bacc(None, num_devices=NCORES, target_bir_lowering=False, debug=True)

    at = nc.declare_dram_parameter("at", [N, NB], bf16, isOutput=False)
    xb = nc.declare_dram_parameter("xb", [N, IN], bf16, isOutput=False)
    w1 = nc.declare_dram_parameter("w1", [IN, F3], bf16, isOutput=False)
    w2 = nc.declare_dram_parameter("w2", [HID, F3], bf16, isOutput=False)
    aw = nc.declare_dram_parameter("aw", [F3, F3], bf16, isOutput=False)
    dwx = nc.declare_dram_parameter("dwx", [HID, IN], bf16, isOutput=False)
    dws = nc.declare_dram_parameter("dws", [HID, IN], bf16, isOutput=False)

    s_rows = nc.declare_dram_parameter("s_rows", [NB, N], f32, isOutput=True)
    x_rows = nc.declare_dram_parameter("x_rows", [NB, IN], f32, isOutput=True)
    att_rows = nc.declare_dram_parameter("att_rows", [NB, F3], f32, isOutput=True)

    ag1_in = nc.dram_tensor("ag1_in", [NB, F3], bf16)
    ag1_out = nc.dram_tensor("ag1_out", [N, F3], bf16, addr_space="Shared")
    ag2_in = nc.dram_tensor("ag2_in", [NB, HID], bf16)
    ag2_out = nc.dram_tensor("ag2_out", [N, HID], bf16, addr_space="Shared")
    ag3_in = nc.dram_tensor("ag3_in", [IN, NB], bf16)
    ag3_out = nc.dram_tensor("ag3_out", [NCORES * IN, NB], bf16, addr_space="Shared")

    def ag(in_t, out_t):
        nc.gpsimd.collective_compute(
            "AllGather",
            mybir.AluOpType.bypass,
            replica_groups=RG,
            ins=[in_t[:, :]],
            outs=[out_t[:, :]],
        )

    with TileContext(nc) as tc:
        with (
            tc.tile_pool(name="pat", bufs=1) as pat,
            tc.tile_pool(name="ph", bufs=1) as ph,
            tc.tile_pool(name="pw", bufs=1) as pw,
            tc.tile_pool(name="psm", bufs=1) as psm,
            tc.tile_pool(name="pacc", bufs=1, space="PSUM") as pacc,
            tc.tile_pool(name="prot", bufs=2, space="PSUM") as prot,
            tc.tile_pool(name="pk", bufs=2, space="PSUM") as pk,
        ):
            # ---- weights
            w1_sb = pw.tile([IN, F3], bf16, name="w1_sb")
            nc.sync.dma_start(out=w1_sb, in_=w1[:, :])
            w2_sb = pw.tile([HID, F3], bf16, name="w2_sb")
            nc.sync.dma_start(out=w2_sb, in_=w2[:, :])
            aw_hi = pw.tile([P, F3], bf16, name="aw_hi")
            nc.sync.dma_start(out=aw_hi, in_=aw[0:P, :])
            aw_lo = pw.tile([F3 - P, F3], bf16, name="aw_lo")
            nc.sync.dma_start(out=aw_lo, in_=aw[P:F3, :])
            dwx_sb = pw.tile([HID, IN], bf16, name="dwx_sb")
            nc.sync.dma_start(out=dwx_sb, in_=dwx[:, :])
            dws_sb = pw.tile([HID, IN], bf16, name="dws_sb")
            nc.sync.dma_start(out=dws_sb, in_=dws[:, :])

            # ---- stage A: adjacency + x loads, SpMM1: a0T = (A_hat x)^T
            at_sb = pat.tile([P, KC, NB], bf16, name="at_sb")
            for g in range(16):
                nc.sync.dma_start(
                    out=at_sb[:, g * 4 : (g + 1) * 4, :],
                    in_=at[g * 512 : (g + 1) * 512, :].rearrange(
                        "(c p) j -> p c j", p=P
                    ),
                )
            x_sb = ph.tile([P, KC, IN], bf16, name="x_sb", tag="hbuf")
            nc.sync.dma_start(out=x_sb, in_=xb.rearrange("(c p) f -> p c f", p=P))

            a0_ps0 = pacc.tile([IN, 512], f32, name="a0_ps0", tag="accA")
            a0_ps1 = pacc.tile([IN, 512], f32, name="a0_ps1", tag="accB")
            for k in range(KC):
                st, sp = (k == 0), (k == KC - 1)
                nc.tensor.matmul(
                    a0_ps0, x_sb[:, k, :], at_sb[:, k, 0:512], start=st, stop=sp
                )
                nc.tensor.matmul(
                    a0_ps1, x_sb[:, k, :], at_sb[:, k, 512:1024], start=st, stop=sp
                )
            a0_sb = psm.tile([IN, NB], bf16, name="a0_sb")
            nc.vector.tensor_copy(out=a0_sb[:, 0:512], in_=a0_ps0)
            nc.vector.tensor_copy(out=a0_sb[:, 512:1024], in_=a0_ps1)

            # ---- stage B: H1 = relu(a0 @ W1cat), node-major
            h1_sb = psm.tile([P, MT, F3], bf16, name="h1_sb")
            for m in range(MT):
                ps = prot.tile([P, F3], f32, name="h1_ps", tag="rot")
                nc.tensor.matmul(
                    ps, a0_sb[:, ds(m * P, P)], w1_sb, start=True, stop=True
                )
                nc.scalar.activation(h1_sb[:, m, :], ps, Relu)
            nc.sync.dma_start(
                out=ag1_in.rearrange("(m p) f -> p m f", p=P), in_=h1_sb
            )

            # ---- AG1 + reload node-major H1 (all nodes)
            ag(ag1_in, ag1_out)
            H1_sb = ph.tile([P, KC, F3], bf16, name="H1_sb", tag="hbuf")
            nc.sync.dma_start(
                out=H1_sb, in_=ag1_out.rearrange("(c p) f -> p c f", p=P)
            )

            # ---- stage D: SpMM2: a1T = (A_hat H1)^T, two stationary pieces
            a1h0 = pacc.tile([P, 512], f32, name="a1h0", tag="accA")
            a1h1 = pacc.tile([P, 512], f32, name="a1h1", tag="accB")
            a1l0 = pacc.tile([F3 - P, 512], f32, name="a1l0", tag="accC")
            a1l1 = pacc.tile([F3 - P, 512], f32, name="a1l1", tag="accD")
            for k in range(KC):
                st, sp = (k == 0), (k == KC - 1)
                hi = H1_sb[:, k, 0:P]
                lo = H1_sb[:, k, P:F3]
                nc.tensor.matmul(a1h0, hi, at_sb[:, k, 0:512], start=st, stop=sp)
                nc.tensor.matmul(a1h1, hi, at_sb[:, k, 512:1024], start=st, stop=sp)
                nc.tensor.matmul(a1l0, lo, at_sb[:, k, 0:512], start=st, stop=sp)
                nc.tensor.matmul(a1l1, lo, at_sb[:, k, 512:1024], start=st, stop=sp)
            # evacuate into per-encoder base-0 tiles (partition-shifted copies)
            a1_sb = [
                psm.tile([IN, NB], bf16, name=f"a1_sb{e}", tag=f"a1_sb{e}")
                for e in range(3)
            ]
            nc.vector.tensor_copy(out=a1_sb[0][:, 0:512], in_=a1h0[0:64, :])
            nc.vector.tensor_copy(out=a1_sb[0][:, 512:1024], in_=a1h1[0:64, :])
            nc.vector.tensor_copy(out=a1_sb[1][:, 0:512], in_=a1h0[64:128, :])
            nc.vector.tensor_copy(out=a1_sb[1][:, 512:1024], in_=a1h1[64:128, :])
            nc.vector.tensor_copy(out=a1_sb[2][:, 0:512], in_=a1l0)
            nc.vector.tensor_copy(out=a1_sb[2][:, 512:1024], in_=a1l1)

            def a1_enc(e):
                # feature-major agg1 slice for encoder e: [64, NB], base 0
                return a1_sb[e][:, :]

            # ---- stage E: cT (feature-major relu'd concat) + he (node-major)
            cT_hi = psm.tile([P, NB], bf16, name="cT_hi")
            cT_lo = psm.tile([F3 - P, NB], bf16, name="cT_lo")
            for e in range(3):
                for i in range(2):
                    ps = prot.tile([IN, 512], f32, name="ct_ps", tag="rot")
                    nc.tensor.matmul(
                        ps,
                        w2_sb[:, ds(e * HID, HID)],
                        a1_enc(e)[:, ds(i * 512, 512)],
                        start=True,
                        stop=True,
                    )
                    if e == 0:
                        dst = cT_hi[0:64, ds(i * 512, 512)]
                    elif e == 1:
                        dst = cT_hi[64:128, ds(i * 512, 512)]
                    else:
                        dst = cT_lo[0:64, ds(i * 512, 512)]
                    # partition-shifted relu evac (base 0 -> base 64 for e=1)
                    nc.vector.tensor_relu(out=dst, in_=ps)

            he_sb = psm.tile([P, 3, MT, IN], f32, name="he_sb")
            for e in range(3):
                for m in range(MT):
                    ps = prot.tile([P, IN], f32, name="he_ps", tag="rot")
                    nc.tensor.matmul(
                        ps,
                        a1_enc(e)[:, ds(m * P, P)],
                        w2_sb[:, ds(e * HID, HID)],
                        start=True,
                        stop=True,
                    )
                    nc.scalar.activation(he_sb[:, e, m, :], ps, Relu)

            # att_in = c @ att_W, node-major, evacuated to att_sb
            att_sb = psm.tile([P, MT, F3], f32, name="att_sb")
            for m in range(MT):
                ps = prot.tile([P, F3], f32, name="att_ps", tag="rot")
                nc.tensor.matmul(
                    ps, cT_hi[:, ds(m * P, P)], aw_hi, start=True, stop=False
                )
                nc.tensor.matmul(
                    ps, cT_lo[:, ds(m * P, P)], aw_lo, start=False, stop=True
                )
                nc.scalar.activation(att_sb[:, m, :], ps, Copy)

            # ---- stage F: softmax over j (groups of 3) in place, then fuse
            attv = att_sb.rearrange("p m (h j) -> p m j h", j=3)
            mx = psm.tile([P, MT, IN], f32, name="mx", tag="ftmp", bufs=3)
            nc.vector.tensor_max(out=mx, in0=attv[:, :, 0, :], in1=attv[:, :, 1, :])
            nc.vector.tensor_max(out=mx, in0=mx, in1=attv[:, :, 2, :])
            for j in range(3):
                nc.vector.tensor_sub(
                    out=attv[:, :, j, :], in0=attv[:, :, j, :], in1=mx
                )
            for j in range(3):
                nc.scalar.activation(attv[:, :, j, :], attv[:, :, j, :], Exp)
            ssum = psm.tile([P, MT, IN], f32, name="ssum", tag="ftmp", bufs=3)
            nc.vector.tensor_add(
                out=ssum, in0=attv[:, :, 0, :], in1=attv[:, :, 1, :]
            )
            nc.vector.tensor_add(out=ssum, in0=ssum, in1=attv[:, :, 2, :])
            rcp = psm.tile([P, MT, IN], f32, name="rcp", tag="ftmp", bufs=3)
            nc.vector.reciprocal(out=rcp, in_=ssum)
            for j in range(3):
                nc.vector.tensor_mul(
                    out=attv[:, :, j, :], in0=attv[:, :, j, :], in1=rcp
                )
            nc.sync.dma_start(
                out=att_rows.rearrange("(m p) f -> p m f", p=P), in_=att_sb
            )

            hacc = psm.tile([P, MT, IN], f32, name="hacc", tag="ftmp", bufs=3)
            htmp = psm.tile([P, MT, IN], f32, name="htmp", tag="ftmp", bufs=3)
            nc.vector.tensor_mul(out=hacc, in0=he_sb[:, 0], in1=attv[:, :, 0, :])
            nc.vector.tensor_mul(out=htmp, in0=he_sb[:, 1], in1=attv[:, :, 1, :])
            nc.vector.tensor_add(out=hacc, in0=hacc, in1=htmp)
            nc.vector.tensor_mul(out=htmp, in0=he_sb[:, 2], in1=attv[:, :, 2, :])
            h_sb = psm.tile([P, MT, IN], bf16, name="h_sb")
            nc.vector.tensor_add(out=h_sb, in0=hacc, in1=htmp)
            nc.sync.dma_start(
                out=ag2_in.rearrange("(m p) f -> p m f", p=P), in_=h_sb
            )

            # ---- AG2 + reload
            ag(ag2_in, ag2_out)
            H2_sb = ph.tile([P, KC, IN], bf16, name="H2_sb", tag="hbuf")
            nc.sync.dma_start(
                out=H2_sb, in_=ag2_out.rearrange("(c p) f -> p c f", p=P)
            )

            # ---- stage H: SpMM3: a2T = (A_hat h)^T
            a2_ps0 = pacc.tile([IN, 512], f32, name="a2_ps0", tag="accA")
            a2_ps1 = pacc.tile([IN, 512], f32, name="a2_ps1", tag="accB")
            for k in range(KC):
                st, sp = (k == 0), (k == KC - 1)
                nc.tensor.matmul(
                    a2_ps0, H2_sb[:, k, :], at_sb[:, k, 0:512], start=st, stop=sp
                )
                nc.tensor.matmul(
                    a2_ps1, H2_sb[:, k, :], at_sb[:, k, 512:1024], start=st, stop=sp
                )
            a2_sb = psm.tile([IN, NB], bf16, name="a2_sb")
            nc.vector.tensor_copy(out=a2_sb[:, 0:512], in_=a2_ps0)
            nc.vector.tensor_copy(out=a2_sb[:, 512:1024], in_=a2_ps1)

            # ---- stage I: decoder heads
            xo_sb = psm.tile([P, MT, IN], f32, name="xo_sb")
            for m in range(MT):
                ps = prot.tile([P, IN], f32, name="xo_ps", tag="rot")
                nc.tensor.matmul(
                    ps, a2_sb[:, ds(m * P, P)], dwx_sb, start=True, stop=True
                )
                nc.scalar.activation(xo_sb[:, m, :], ps, Copy)
            nc.sync.dma_start(
                out=x_rows.rearrange("(m p) f -> p m f", p=P), in_=xo_sb
            )

            hT_ps0 = pacc.tile([IN, 512], f32, name="hT_ps0", tag="accA")
            hT_ps1 = pacc.tile([IN, 512], f32, name="hT_ps1", tag="accB")
            nc.tensor.matmul(hT_ps0, dws_sb, a2_sb[:, 0:512], start=True, stop=True)
            nc.tensor.matmul(
                hT_ps1, dws_sb, a2_sb[:, 512:1024], start=True, stop=True
            )
            hT_sb = psm.tile([IN, NB], bf16, name="hT_sb")
            nc.vector.tensor_copy(out=hT_sb[:, 0:512], in_=hT_ps0)
            nc.vector.tensor_copy(out=hT_sb[:, 512:1024], in_=hT_ps1)
            nc.sync.dma_start(out=ag3_in[:, :], in_=hT_sb)

            # ---- AG3 + reload h_^T for all nodes
            ag(ag3_in, ag3_out)
            hTf_sb = ph.tile([IN, NCORES, NB], bf16, name="hTf_sb", tag="hbuf")
            nc.sync.dma_start(
                out=hTf_sb, in_=ag3_out.rearrange("(r f) m -> f r m", f=IN)
            )

            # ---- stage K: s_ rows = h_rows @ h_full^T (32 MB f32 out)
            dma_engines = [nc.sync, nc.scalar, nc.gpsimd]
            for m in range(MT):
                lhsT = hT_sb[:, ds(m * P, P)]
                for n in range(16):
                    ps = pk.tile([P, 512], f32, name="s_ps", tag="spk")
                    nc.tensor.matmul(
                        ps,
                        lhsT,
                        hTf_sb[:, n // 2, ds((n % 2) * 512, 512)],
                        start=True,
                        stop=True,
                    )
                    ev = psm.tile([P, 512], f32, name="s_ev", tag="sev", bufs=4)
                    if n % 2 == 0:
                        nc.scalar.copy(out=ev, in_=ps)
                    else:
                        nc.vector.tensor_copy(out=ev, in_=ps)
                    dma_engines[n % 3].dma_start(
                        out=s_rows[ds(m * P, P), ds(n * 512, 512)], in_=ev
                    )

    nc.finalize()
    return nc


def _get_program():
    global _PROG
    if _PROG is None:
        _PROG = _build_program()
    return _PROG


def kernel(**inputs) -> tuple:
    global LAST_EXEC_NS
    import ml_dtypes

    from concourse.bass_utils import run_bass_kernel_spmd

    bf = ml_dtypes.bfloat16

    x = np.asarray(inputs["x"], dtype=np.float32)
    src = np.asarray(inputs["src"]).astype(np.int64)
    dst = np.asarray(inputs["dst"]).astype(np.int64)

    # ---- host-side: bake the normalized adjacency (transposed), per hint:
    # edge partitioning by dst == column shards of A_hat^T.
    deg = 1.0 + np.bincount(dst, minlength=N).astype(np.float64)
    dinv = (1.0 / np.sqrt(deg)).astype(np.float32)
    coef = (dinv[src] * dinv[dst]).astype(np.float64)
    flat = np.bincount(src * N + dst, weights=coef, minlength=N * N)
    AT = flat.astype(np.float32).reshape(N, N)
    AT[np.arange(N), np.arange(N)] += dinv * dinv

    w1 = np.concatenate(
        [inputs["enc_a_W1"], inputs["enc_s_W1"], inputs["enc_t_W1"]], axis=1
    )
    w2 = np.concatenate(
        [inputs["enc_a_W2"], inputs["enc_s_W2"], inputs["enc_t_W2"]], axis=1
    )
    common = {
        "xb": np.ascontiguousarray(x.astype(bf)),
        "w1": np.ascontiguousarray(np.asarray(w1, np.float32).astype(bf)),
        "w2": np.ascontiguousarray(np.asarray(w2, np.float32).astype(bf)),
        "aw": np.ascontiguousarray(np.asarray(inputs["att_W"], np.float32).astype(bf)),
        "dwx": np.ascontiguousarray(
            np.asarray(inputs["dec_x_W"], np.float32).astype(bf)
        ),
        "dws": np.ascontiguousarray(
            np.asarray(inputs["dec_s_W"], np.float32).astype(bf)
        ),
    }
    in_maps = []
    for r in range(NCORES):
        m = dict(common)
        m["at"] = np.ascontiguousarray(AT[:, r * NB : (r + 1) * NB].astype(bf))
        in_maps.append(m)

    nc = _get_program()
    kwargs = {}
    if TRACE:
        kwargs = dict(trace=True, trace_cores=list(range(NCORES)))
    res = run_bass_kernel_spmd(nc, in_maps, core_ids=list(range(NCORES)), **kwargs)
    LAST_EXEC_NS = res.exec_time_ns
    results = res.results

    s_ = np.concatenate([results[r]["s_rows"] for r in range(NCORES)], axis=0)
    x_ = np.concatenate([results[r]["x_rows"] for r in range(NCORES)], axis=0)
    att = np.concatenate(
        [results[r]["att_rows"] for r in range(NCORES)], axis=0
    ).reshape(N, HID, 3)
    return (
        np.asarray(x_, np.float32),
        np.asarray(s_, np.float32),
        np.asarray(att, np.float32),
    )


# revision 8
# speedup vs baseline: 1.2420x; 1.2420x over previous
"""Distributed Trainium2 kernel for the ADAGAD GNN message-passing model.

Model (see problem reference): three 2-layer GCN encoders over a shared
graph, attention-softmax fusion of the three embeddings, two GCN decoder
heads, and a final dense similarity matrix s_ = h_ @ h_.T.

Every GCN conv uses the same symmetric-normalized adjacency with self
loops, A_hat = D^-1/2 (A + I) D^-1/2 (D = 1 + in-degree).  The host
pre-bakes A_hat^T densely in bf16 and column-shards it over the 8 cores
(columns = destination nodes, matching the "partition edges by dst"
sharding).  Each core keeps its 16 MB shard resident in SBUF and runs all
sparse aggregations as dense TensorE matmuls in "outT" form:

    aggT[f, m] = sum_k H[k, f] * A_hatT[k, m]   (lhsT = H chunk, rhs = A_hatT)

which produces feature-major aggregates whose slices feed directly as
lhsT into the small dense-weight matmuls, flipping back to node-major
with no transposes anywhere.  Cross-core exchange is three bf16
AllGathers (H1 after encoder layer 1, h after fusion, h_^T before the
final row-sharded h_ @ h_.T whose 32 MB/core f32 output write is the
memory-roofline term).
"""

import numpy as np

N = 8192
IN = 64
HID = 64
F3 = 3 * HID          # 192
NCORES = 8
NB = N // NCORES      # 1024 rows (dst nodes) per core
P = 128               # partitions
KC = N // P           # 64 contraction chunks
MT = NB // P          # 8 m-tiles per core

TRACE = False         # set by test harness to collect HW exec time
LAST_EXEC_NS = None

_PROG = None


def _build_program():
    import concourse.bass as bass
    import concourse.mybir as mybir
    from concourse import bacc
    from concourse.bass import ds
    from concourse.tile import TileContext

    bf16 = mybir.dt.bfloat16
    f32 = mybir.dt.float32
    Relu = mybir.ActivationFunctionType.Relu
    Exp = mybir.ActivationFunctionType.Exp
    Copy = mybir.ActivationFunctionType.Copy
    RG = [list(range(NCORES))]

    nc = bacc.Bacc(None, num_devices=NCORES, target_bir_lowering=False, debug=True)

    # host pre-arranged to SBUF layout: [p, k, :] = row k*128+p of the
    # node-major matrix, flattened -> fully contiguous per-partition DMAs
    at = nc.declare_dram_parameter("at", [P, KC * NB], bf16, isOutput=False)
    xb = nc.declare_dram_parameter("xb", [P, KC * IN], bf16, isOutput=False)
    w1 = nc.declare_dram_parameter("w1", [IN, F3], bf16, isOutput=False)
    w2 = nc.declare_dram_parameter("w2", [HID, F3], bf16, isOutput=False)
    aw = nc.declare_dram_parameter("aw", [F3, F3], bf16, isOutput=False)
    dwx = nc.declare_dram_parameter("dwx", [HID, IN], bf16, isOutput=False)
    dws = nc.declare_dram_parameter("dws", [HID, IN], bf16, isOutput=False)

    s_rows = nc.declare_dram_parameter("s_rows", [NB, N], f32, isOutput=True)
    x_rows = nc.declare_dram_parameter("x_rows", [NB, IN], f32, isOutput=True)
    att_rows = nc.declare_dram_parameter("att_rows", [NB, F3], f32, isOutput=True)

    # AG1/AG2 bounces keep the SBUF tile layout: in = [P, MT*F], out adds a
    # leading rank dim; global chunk k = r*MT + m matches at_sb row order.
    ag1_in = nc.dram_tensor("ag1_in", [P, MT * F3], bf16)
    ag1_out = nc.dram_tensor("ag1_out", [NCORES, P, MT * F3], bf16, addr_space="Shared")
    ag2_in = nc.dram_tensor("ag2_in", [P, MT * HID], bf16)
    ag2_out = nc.dram_tensor("ag2_out", [NCORES, P, MT * HID], bf16, addr_space="Shared")
    ag3_in = nc.dram_tensor("ag3_in", [IN, NB], bf16)
    ag3_out = nc.dram_tensor("ag3_out", [NCORES * IN, NB], bf16, addr_space="Shared")

    def ag(in_t, out_t):
        nc.gpsimd.collective_compute(
            "AllGather",
            mybir.AluOpType.bypass,
            replica_groups=RG,
            ins=[in_t[...]],
            outs=[out_t[...]],
        )

    with TileContext(nc) as tc:
        with (
            tc.tile_pool(name="pat", bufs=1) as pat,
            tc.tile_pool(name="ph", bufs=1) as ph,
            tc.tile_pool(name="pw", bufs=1) as pw,
            tc.tile_pool(name="psm", bufs=1) as psm,
            tc.tile_pool(name="pacc", bufs=1, space="PSUM") as pacc,
            tc.tile_pool(name="prot", bufs=1, space="PSUM") as prot,
            tc.tile_pool(name="pk", bufs=3, space="PSUM") as pk,
        ):
            # ---- weights
            w1_sb = pw.tile([IN, F3], bf16, name="w1_sb")
            nc.sync.dma_start(out=w1_sb, in_=w1[:, :])
            w2_sb = pw.tile([HID, F3], bf16, name="w2_sb")
            nc.sync.dma_start(out=w2_sb, in_=w2[:, :])
            aw_hi = pw.tile([P, F3], bf16, name="aw_hi")
            nc.sync.dma_start(out=aw_hi, in_=aw[0:P, :])
            aw_lo = pw.tile([F3 - P, F3], bf16, name="aw_lo")
            nc.sync.dma_start(out=aw_lo, in_=aw[P:F3, :])
            dwx_sb = pw.tile([HID, IN], bf16, name="dwx_sb")
            nc.sync.dma_start(out=dwx_sb, in_=dwx[:, :])
            dws_sb = pw.tile([HID, IN], bf16, name="dws_sb")
            nc.sync.dma_start(out=dws_sb, in_=dws[:, :])

            # ---- stage A: adjacency + x loads, SpMM1: a0T = (A_hat x)^T
            at_sb = pat.tile([P, KC, NB], bf16, name="at_sb")
            for g in range(16):
                nc.sync.dma_start(
                    out=at_sb[:, g * 4 : (g + 1) * 4, :],
                    in_=at[:, g * 4 * NB : (g + 1) * 4 * NB],
                )
            x_sb = ph.tile([P, KC, IN], bf16, name="x_sb", tag="hbuf")
            nc.sync.dma_start(out=x_sb, in_=xb[:, :])

            a0_ps0 = pacc.tile([IN, 512], f32, name="a0_ps0", tag="accA")
            a0_ps1 = pacc.tile([IN, 512], f32, name="a0_ps1", tag="accB")
            for k in range(KC):
                st, sp = (k == 0), (k == KC - 1)
                nc.tensor.matmul(
                    a0_ps0, x_sb[:, k, :], at_sb[:, k, 0:512], start=st, stop=sp
                )
                nc.tensor.matmul(
                    a0_ps1, x_sb[:, k, :], at_sb[:, k, 512:1024], start=st, stop=sp
                )
            a0_sb = psm.tile([IN, NB], bf16, name="a0_sb")
            nc.vector.tensor_copy(out=a0_sb[:, 0:512], in_=a0_ps0)
            nc.vector.tensor_copy(out=a0_sb[:, 512:1024], in_=a0_ps1)

            # ---- stage B: H1 = relu(a0 @ W1cat), node-major
            h1_sb = psm.tile([P, MT, F3], bf16, name="h1_sb")
            for m in range(MT):
                ps = prot.tile([P, F3], f32, name="h1_ps", tag="rot")
                nc.tensor.matmul(
                    ps, a0_sb[:, ds(m * P, P)], w1_sb, start=True, stop=True
                )
                nc.scalar.activation(h1_sb[:, m, :], ps, Relu)
            nc.sync.dma_start(out=ag1_in[:, :], in_=h1_sb)

            # ---- AG1 + reload node-major H1 (all nodes)
            ag(ag1_in, ag1_out)
            H1_sb = ph.tile([P, KC, F3], bf16, name="H1_sb", tag="hbuf")
            nc.sync.dma_start(
                out=H1_sb, in_=ag1_out.rearrange("r p q -> p r q")
            )

            # ---- stage D: SpMM2: a1T = (A_hat H1)^T, two stationary pieces
            a1h0 = pacc.tile([P, 512], f32, name="a1h0", tag="accA")
            a1h1 = pacc.tile([P, 512], f32, name="a1h1", tag="accB")
            a1l0 = pacc.tile([F3 - P, 512], f32, name="a1l0", tag="accC")
            a1l1 = pacc.tile([F3 - P, 512], f32, name="a1l1", tag="accD")
            for k in range(KC):
                st, sp = (k == 0), (k == KC - 1)
                hi = H1_sb[:, k, 0:P]
                lo = H1_sb[:, k, P:F3]
                nc.tensor.matmul(a1h0, hi, at_sb[:, k, 0:512], start=st, stop=sp)
                nc.tensor.matmul(a1h1, hi, at_sb[:, k, 512:1024], start=st, stop=sp)
                nc.tensor.matmul(a1l0, lo, at_sb[:, k, 0:512], start=st, stop=sp)
                nc.tensor.matmul(a1l1, lo, at_sb[:, k, 512:1024], start=st, stop=sp)
            # evacuate into per-encoder base-0 tiles (partition-shifted copies)
            a1_sb = [
                psm.tile([IN, NB], bf16, name=f"a1_sb{e}", tag=f"a1_sb{e}")
                for e in range(3)
            ]
            nc.vector.tensor_copy(out=a1_sb[0][:, 0:512], in_=a1h0[0:64, :])
            nc.vector.tensor_copy(out=a1_sb[0][:, 512:1024], in_=a1h1[0:64, :])
            nc.vector.tensor_copy(out=a1_sb[1][:, 0:512], in_=a1h0[64:128, :])
            nc.vector.tensor_copy(out=a1_sb[1][:, 512:1024], in_=a1h1[64:128, :])
            nc.vector.tensor_copy(out=a1_sb[2][:, 0:512], in_=a1l0)
            nc.vector.tensor_copy(out=a1_sb[2][:, 512:1024], in_=a1l1)

            def a1_enc(e):
                # feature-major agg1 slice for encoder e: [64, NB], base 0
                return a1_sb[e][:, :]

            # ---- stage E: cT (feature-major relu'd concat) + he (node-major)
            cT_hi = psm.tile([P, NB], bf16, name="cT_hi")
            cT_lo = psm.tile([F3 - P, NB], bf16, name="cT_lo")
            for e in range(3):
                for i in range(2):
                    ps = prot.tile([IN, 512], f32, name="ct_ps", tag="rot")
                    nc.tensor.matmul(
                        ps,
                        w2_sb[:, ds(e * HID, HID)],
                        a1_enc(e)[:, ds(i * 512, 512)],
                        start=True,
                        stop=True,
                    )
                    if e == 0:
                        dst = cT_hi[0:64, ds(i * 512, 512)]
                    elif e == 1:
                        dst = cT_hi[64:128, ds(i * 512, 512)]
                    else:
                        dst = cT_lo[0:64, ds(i * 512, 512)]
                    # partition-shifted relu evac (base 0 -> base 64 for e=1)
                    nc.vector.tensor_relu(out=dst, in_=ps)

            he_sb = psm.tile([P, 3, MT, IN], f32, name="he_sb")
            for e in range(3):
                for m in range(MT):
                    ps = prot.tile([P, IN], f32, name="he_ps", tag="rot")
                    nc.tensor.matmul(
                        ps,
                        a1_enc(e)[:, ds(m * P, P)],
                        w2_sb[:, ds(e * HID, HID)],
                        start=True,
                        stop=True,
                    )
                    nc.scalar.activation(he_sb[:, e, m, :], ps, Relu)

            # att_in = c @ att_W, node-major, evacuated to att_sb
            att_sb = psm.tile([P, MT, F3], f32, name="att_sb")
            for m in range(MT):
                ps = prot.tile([P, F3], f32, name="att_ps", tag="rot")
                nc.tensor.matmul(
                    ps, cT_hi[:, ds(m * P, P)], aw_hi, start=True, stop=False
                )
                nc.tensor.matmul(
                    ps, cT_lo[:, ds(m * P, P)], aw_lo, start=False, stop=True
                )
                nc.scalar.activation(att_sb[:, m, :], ps, Copy)

            # ---- stage F: softmax over j (groups of 3) in place, then fuse
            attv = att_sb.rearrange("p m (h j) -> p m j h", j=3)
            mx = psm.tile([P, MT, IN], f32, name="mx", tag="ftmp", bufs=3)
            nc.vector.tensor_max(out=mx, in0=attv[:, :, 0, :], in1=attv[:, :, 1, :])
            nc.vector.tensor_max(out=mx, in0=mx, in1=attv[:, :, 2, :])
            for j in range(3):
                nc.vector.tensor_sub(
                    out=attv[:, :, j, :], in0=attv[:, :, j, :], in1=mx
                )
            for j in range(3):
                nc.scalar.activation(attv[:, :, j, :], attv[:, :, j, :], Exp)
            ssum = psm.tile([P, MT, IN], f32, name="ssum", tag="ftmp", bufs=3)
            nc.vector.tensor_add(
                out=ssum, in0=attv[:, :, 0, :], in1=attv[:, :, 1, :]
            )
            nc.vector.tensor_add(out=ssum, in0=ssum, in1=attv[:, :, 2, :])
            rcp = psm.tile([P, MT, IN], f32, name="rcp", tag="ftmp", bufs=3)
            nc.vector.reciprocal(out=rcp, in_=ssum)
            for j in range(3):
                nc.vector.tensor_mul(
                    out=attv[:, :, j, :], in0=attv[:, :, j, :], in1=rcp
                )
            nc.sync.dma_start(
                out=att_rows.rearrange("(m p) f -> p m f", p=P), in_=att_sb
            )

            hacc = psm.tile([P, MT, IN], f32, name="hacc", tag="ftmp", bufs=3)
            htmp = psm.tile([P, MT, IN], f32, name="htmp", tag="ftmp", bufs=3)
            nc.vector.tensor_mul(out=hacc, in0=he_sb[:, 0], in1=attv[:, :, 0, :])
            nc.vector.tensor_mul(out=htmp, in0=he_sb[:, 1], in1=attv[:, :, 1, :])
            nc.vector.tensor_add(out=hacc, in0=hacc, in1=htmp)
            nc.vector.tensor_mul(out=htmp, in0=he_sb[:, 2], in1=attv[:, :, 2, :])
            h_sb = psm.tile([P, MT, IN], bf16, name="h_sb")
            nc.vector.tensor_add(out=h_sb, in0=hacc, in1=htmp)
            nc.sync.dma_start(out=ag2_in[:, :], in_=h_sb)

            # ---- AG2 + reload
            ag(ag2_in, ag2_out)
            H2_sb = ph.tile([P, KC, IN], bf16, name="H2_sb", tag="hbuf")
            nc.sync.dma_start(
                out=H2_sb, in_=ag2_out.rearrange("r p q -> p r q")
            )

            # ---- stage H: SpMM3: a2T = (A_hat h)^T
            a2_ps0 = pacc.tile([IN, 512], f32, name="a2_ps0", tag="accA")
            a2_ps1 = pacc.tile([IN, 512], f32, name="a2_ps1", tag="accB")
            for k in range(KC):
                st, sp = (k == 0), (k == KC - 1)
                nc.tensor.matmul(
                    a2_ps0, H2_sb[:, k, :], at_sb[:, k, 0:512], start=st, stop=sp
                )
                nc.tensor.matmul(
                    a2_ps1, H2_sb[:, k, :], at_sb[:, k, 512:1024], start=st, stop=sp
                )
            a2_sb = psm.tile([IN, NB], bf16, name="a2_sb")
            nc.vector.tensor_copy(out=a2_sb[:, 0:512], in_=a2_ps0)
            nc.vector.tensor_copy(out=a2_sb[:, 512:1024], in_=a2_ps1)

            # ---- stage I: decoder heads
            xo_sb = psm.tile([P, MT, IN], f32, name="xo_sb")
            for m in range(MT):
                ps = prot.tile([P, IN], f32, name="xo_ps", tag="rot")
                nc.tensor.matmul(
                    ps, a2_sb[:, ds(m * P, P)], dwx_sb, start=True, stop=True
                )
                nc.scalar.activation(xo_sb[:, m, :], ps, Copy)
            nc.sync.dma_start(
                out=x_rows.rearrange("(m p) f -> p m f", p=P), in_=xo_sb
            )

            hT_ps0 = pacc.tile([IN, 512], f32, name="hT_ps0", tag="accA")
            hT_ps1 = pacc.tile([IN, 512], f32, name="hT_ps1", tag="accB")
            nc.tensor.matmul(hT_ps0, dws_sb, a2_sb[:, 0:512], start=True, stop=True)
            nc.tensor.matmul(
                hT_ps1, dws_sb, a2_sb[:, 512:1024], start=True, stop=True
            )
            hT_sb = psm.tile([IN, NB], bf16, name="hT_sb")
            nc.vector.tensor_copy(out=hT_sb[:, 0:512], in_=hT_ps0)
            nc.vector.tensor_copy(out=hT_sb[:, 512:1024], in_=hT_ps1)
            nc.sync.dma_start(out=ag3_in[:, :], in_=hT_sb)

            # ---- AG3 + reload h_^T for all nodes
            ag(ag3_in, ag3_out)
            hTf_sb = ph.tile([IN, NCORES, NB], bf16, name="hTf_sb", tag="hbuf")
            nc.sync.dma_start(
                out=hTf_sb, in_=ag3_out.rearrange("(r f) m -> f r m", f=IN)
            )

            # ---- stage K: s_ rows = h_rows @ h_full^T (32 MB f32 out)
            dma_engines = [nc.sync, nc.scalar, nc.gpsimd]
            for m in range(MT):
                lhsT = hT_sb[:, ds(m * P, P)]
                for g in range(8):  # groups of 2 n-tiles -> one 512 KB DMA
                    ev = psm.tile([P, 1024], f32, name="s_ev", tag="sev", bufs=3)
                    for i in range(2):
                        n = g * 2 + i
                        ps = pk.tile([P, 512], f32, name="s_ps", tag="spk")
                        nc.tensor.matmul(
                            ps,
                            lhsT,
                            hTf_sb[:, n // 2, ds((n % 2) * 512, 512)],
                            start=True,
                            stop=True,
                        )
                        if i % 2 == 0:
                            nc.scalar.copy(out=ev[:, ds(i * 512, 512)], in_=ps)
                        else:
                            nc.vector.tensor_copy(
                                out=ev[:, ds(i * 512, 512)], in_=ps
                            )
                    dma_engines[g % 3].dma_start(
                        out=s_rows[ds(m * P, P), ds(g * 1024, 1024)], in_=ev
                    )

    nc.finalize()
    return nc


def _get_program():
    global _PROG
    if _PROG is None:
        _PROG = _build_program()
    return _PROG


def kernel(**inputs) -> tuple:
    global LAST_EXEC_NS
    import ml_dtypes

    from concourse.bass_utils import run_bass_kernel_spmd

    bf = ml_dtypes.bfloat16

    x = np.asarray(inputs["x"], dtype=np.float32)
    src = np.asarray(inputs["src"]).astype(np.int64)
    dst = np.asarray(inputs["dst"]).astype(np.int64)

    # ---- host-side: bake the normalized adjacency (transposed), per hint:
    # edge partitioning by dst == column shards of A_hat^T.
    deg = 1.0 + np.bincount(dst, minlength=N).astype(np.float64)
    dinv = (1.0 / np.sqrt(deg)).astype(np.float32)
    coef = (dinv[src] * dinv[dst]).astype(np.float64)
    flat = np.bincount(src * N + dst, weights=coef, minlength=N * N)
    AT = flat.astype(np.float32).reshape(N, N)
    AT[np.arange(N), np.arange(N)] += dinv * dinv

    w1 = np.concatenate(
        [inputs["enc_a_W1"], inputs["enc_s_W1"], inputs["enc_t_W1"]], axis=1
    )
    w2 = np.concatenate(
        [inputs["enc_a_W2"], inputs["enc_s_W2"], inputs["enc_t_W2"]], axis=1
    )
    def to_sbuf_layout(mat):
        # [N, F] node-major -> [128, KC*F]: row p holds chunks k of node k*128+p
        f = mat.shape[1]
        return np.ascontiguousarray(
            mat.reshape(KC, P, f).transpose(1, 0, 2).reshape(P, KC * f)
        )

    common = {
        "xb": to_sbuf_layout(x.astype(bf)),
        "w1": np.ascontiguousarray(np.asarray(w1, np.float32).astype(bf)),
        "w2": np.ascontiguousarray(np.asarray(w2, np.float32).astype(bf)),
        "aw": np.ascontiguousarray(np.asarray(inputs["att_W"], np.float32).astype(bf)),
        "dwx": np.ascontiguousarray(
            np.asarray(inputs["dec_x_W"], np.float32).astype(bf)
        ),
        "dws": np.ascontiguousarray(
            np.asarray(inputs["dec_s_W"], np.float32).astype(bf)
        ),
    }
    in_maps = []
    for r in range(NCORES):
        m = dict(common)
        m["at"] = to_sbuf_layout(AT[:, r * NB : (r + 1) * NB].astype(bf))
        in_maps.append(m)

    nc = _get_program()
    kwargs = {}
    if TRACE:
        kwargs = dict(trace=True, trace_cores=list(range(NCORES)))
    res = run_bass_kernel_spmd(nc, in_maps, core_ids=list(range(NCORES)), **kwargs)
    LAST_EXEC_NS = res.exec_time_ns
    results = res.results

    s_ = np.concatenate([results[r]["s_rows"] for r in range(NCORES)], axis=0)
    x_ = np.concatenate([results[r]["x_rows"] for r in range(NCORES)], axis=0)
    att = np.concatenate(
        [results[r]["att_rows"] for r in range(NCORES)], axis=0
    ).reshape(N, HID, 3)
    return (
        np.asarray(x_, np.float32),
        np.asarray(s_, np.float32),
        np.asarray(att, np.float32),
    )


# revision 9
# speedup vs baseline: 1.3964x; 1.1243x over previous
"""Distributed Trainium2 kernel for the ADAGAD GNN message-passing model.

Model (see problem reference): three 2-layer GCN encoders over a shared
graph, attention-softmax fusion of the three embeddings, two GCN decoder
heads, and a final dense similarity matrix s_ = h_ @ h_.T.

Every GCN conv uses the same symmetric-normalized adjacency with self
loops, A_hat = D^-1/2 (A + I) D^-1/2 (D = 1 + in-degree).  The host
pre-bakes A_hat^T densely in bf16 and column-shards it over the 8 cores
(columns = destination nodes, matching the "partition edges by dst"
sharding).  Each core keeps its 16 MB shard resident in SBUF and runs all
sparse aggregations as dense TensorE matmuls in "outT" form:

    aggT[f, m] = sum_k H[k, f] * A_hatT[k, m]   (lhsT = H chunk, rhs = A_hatT)

which produces feature-major aggregates whose slices feed directly as
lhsT into the small dense-weight matmuls, flipping back to node-major
with no transposes anywhere.  Cross-core exchange is three bf16
AllGathers (H1 after encoder layer 1, h after fusion, h_^T before the
final row-sharded h_ @ h_.T whose 32 MB/core f32 output write is the
memory-roofline term).
"""

import numpy as np

N = 8192
IN = 64
HID = 64
F3 = 3 * HID          # 192
NCORES = 8
NB = N // NCORES      # 1024 rows (dst nodes) per core
P = 128               # partitions
KC = N // P           # 64 contraction chunks
MT = NB // P          # 8 m-tiles per core

TRACE = False         # set by test harness to collect HW exec time
LAST_EXEC_NS = None

_PROG = None


def _build_program():
    import concourse.bass as bass
    import concourse.mybir as mybir
    from concourse import bacc
    from concourse.bass import ds
    from concourse.tile import TileContext

    bf16 = mybir.dt.bfloat16
    f32 = mybir.dt.float32
    Relu = mybir.ActivationFunctionType.Relu
    Exp = mybir.ActivationFunctionType.Exp
    Copy = mybir.ActivationFunctionType.Copy
    RG = [list(range(NCORES))]

    nc = bacc.Bacc(None, num_devices=NCORES, target_bir_lowering=False, debug=True)

    # host pre-arranged to SBUF layout: [p, k, :] = row k*128+p of the
    # node-major matrix, flattened -> fully contiguous per-partition DMAs
    at = nc.declare_dram_parameter("at", [P, KC * NB], bf16, isOutput=False)
    xb = nc.declare_dram_parameter("xb", [P, KC * IN], bf16, isOutput=False)
    w1 = nc.declare_dram_parameter("w1", [IN, F3], bf16, isOutput=False)
    w2 = nc.declare_dram_parameter("w2", [HID, F3], bf16, isOutput=False)
    aw = nc.declare_dram_parameter("aw", [F3, F3], bf16, isOutput=False)
    dwx = nc.declare_dram_parameter("dwx", [HID, IN], bf16, isOutput=False)
    dws = nc.declare_dram_parameter("dws", [HID, IN], bf16, isOutput=False)

    s_rows = nc.declare_dram_parameter("s_rows", [NB, N], f32, isOutput=True)
    x_rows = nc.declare_dram_parameter("x_rows", [NB, IN], f32, isOutput=True)
    att_rows = nc.declare_dram_parameter("att_rows", [NB, F3], f32, isOutput=True)

    # AG1/AG2 bounces keep the SBUF tile layout: in = [P, MT*F], out adds a
    # leading rank dim; global chunk k = r*MT + m matches at_sb row order.
    ag1a_in = nc.dram_tensor("ag1a_in", [P, MT * P], bf16)
    ag1a_out = nc.dram_tensor("ag1a_out", [NCORES, P, MT * P], bf16, addr_space="Shared")
    ag1b_in = nc.dram_tensor("ag1b_in", [P, MT * (F3 - P)], bf16)
    ag1b_out = nc.dram_tensor("ag1b_out", [NCORES, P, MT * (F3 - P)], bf16, addr_space="Shared")
    ag2_in = nc.dram_tensor("ag2_in", [P, MT * HID], bf16)
    ag2_out = nc.dram_tensor("ag2_out", [NCORES, P, MT * HID], bf16, addr_space="Shared")
    agw_in = nc.dram_tensor("agw_in", [1, 16], bf16)
    agw_out = nc.dram_tensor("agw_out", [NCORES, 16], bf16, addr_space="Shared")
    ag3_in = nc.dram_tensor("ag3_in", [IN, NB], bf16)
    ag3_out = nc.dram_tensor("ag3_out", [NCORES * IN, NB], bf16, addr_space="Shared")

    def ag(in_t, out_t):
        nc.gpsimd.collective_compute(
            "AllGather",
            mybir.AluOpType.bypass,
            replica_groups=RG,
            ins=[in_t[...]],
            outs=[out_t[...]],
        )

    with TileContext(nc) as tc:
        with (
            tc.tile_pool(name="pat", bufs=1) as pat,
            tc.tile_pool(name="ph", bufs=1) as ph,
            tc.tile_pool(name="pw", bufs=1) as pw,
            tc.tile_pool(name="psm", bufs=1) as psm,
            tc.tile_pool(name="pacc", bufs=1, space="PSUM") as pacc,
            tc.tile_pool(name="prot", bufs=2, space="PSUM") as prot,
        ):
            # warm up the collective path while the big loads stream
            ag(agw_in, agw_out)

            # ---- weights
            w1_sb = pw.tile([IN, F3], bf16, name="w1_sb")
            nc.sync.dma_start(out=w1_sb, in_=w1[:, :])
            w2_sb = pw.tile([HID, F3], bf16, name="w2_sb")
            nc.sync.dma_start(out=w2_sb, in_=w2[:, :])
            aw_hi = pw.tile([P, F3], bf16, name="aw_hi")
            nc.sync.dma_start(out=aw_hi, in_=aw[0:P, :])
            aw_lo = pw.tile([F3 - P, F3], bf16, name="aw_lo")
            nc.sync.dma_start(out=aw_lo, in_=aw[P:F3, :])
            dwx_sb = pw.tile([HID, IN], bf16, name="dwx_sb")
            nc.sync.dma_start(out=dwx_sb, in_=dwx[:, :])
            dws_sb = pw.tile([HID, IN], bf16, name="dws_sb")
            nc.sync.dma_start(out=dws_sb, in_=dws[:, :])

            # ---- stage A: adjacency + x loads, SpMM1: a0T = (A_hat x)^T
            x_sb = ph.tile([P, KC, IN], bf16, name="x_sb", tag="hbuf")
            nc.scalar.dma_start(out=x_sb, in_=xb[:, :])
            at_sb = pat.tile([P, KC, NB], bf16, name="at_sb")
            for g in range(16):
                eng = nc.sync if g % 2 == 0 else nc.scalar
                eng.dma_start(
                    out=at_sb[:, g * 4 : (g + 1) * 4, :],
                    in_=at[:, g * 4 * NB : (g + 1) * 4 * NB],
                )

            a0_ps0 = pacc.tile([IN, 512], f32, name="a0_ps0", tag="accA")
            a0_ps1 = pacc.tile([IN, 512], f32, name="a0_ps1", tag="accB")
            for k in range(KC):
                st, sp = (k == 0), (k == KC - 1)
                nc.tensor.matmul(
                    a0_ps0, x_sb[:, k, :], at_sb[:, k, 0:512], start=st, stop=sp
                )
                nc.tensor.matmul(
                    a0_ps1, x_sb[:, k, :], at_sb[:, k, 512:1024], start=st, stop=sp
                )
            a0_sb = psm.tile([IN, NB], bf16, name="a0_sb")
            nc.vector.tensor_copy(out=a0_sb[:, 0:512], in_=a0_ps0)
            nc.vector.tensor_copy(out=a0_sb[:, 512:1024], in_=a0_ps1)

            # ---- stage B: H1 = relu(a0 @ W1cat), node-major, split hi/lo
            h1a_sb = psm.tile([P, MT, P], bf16, name="h1a_sb")
            h1b_sb = psm.tile([P, MT, F3 - P], bf16, name="h1b_sb")
            for m in range(MT):
                ps = prot.tile([P, F3], f32, name="h1_ps", tag="rot")
                nc.tensor.matmul(
                    ps, a0_sb[:, ds(m * P, P)], w1_sb, start=True, stop=True
                )
                nc.scalar.activation(h1a_sb[:, m, :], ps[:, 0:P], Relu)
                nc.scalar.activation(h1b_sb[:, m, :], ps[:, P:F3], Relu)
            nc.sync.dma_start(out=ag1a_in[:, :], in_=h1a_sb)
            nc.sync.dma_start(out=ag1b_in[:, :], in_=h1b_sb)

            # ---- AG1 (split: hi lands first, D-hi starts while lo flies)
            ag(ag1a_in, ag1a_out)
            ag(ag1b_in, ag1b_out)
            H1a_sb = ph.tile([P, KC, P], bf16, name="H1a_sb", tag="hbuf")
            nc.sync.dma_start(
                out=H1a_sb, in_=ag1a_out.rearrange("r p q -> p r q")
            )
            H1b_sb = ph.tile([P, KC, F3 - P], bf16, name="H1b_sb", tag="hbuf2")
            nc.scalar.dma_start(
                out=H1b_sb, in_=ag1b_out.rearrange("r p q -> p r q")
            )

            # ---- stage D: SpMM2: a1T = (A_hat H1)^T, two stationary pieces
            a1h0 = pacc.tile([P, 512], f32, name="a1h0", tag="accA")
            a1h1 = pacc.tile([P, 512], f32, name="a1h1", tag="accB")
            a1l0 = pacc.tile([F3 - P, 512], f32, name="a1l0", tag="accC")
            a1l1 = pacc.tile([F3 - P, 512], f32, name="a1l1", tag="accD")
            for k in range(KC):
                st, sp = (k == 0), (k == KC - 1)
                hi = H1a_sb[:, k, :]
                nc.tensor.matmul(a1h0, hi, at_sb[:, k, 0:512], start=st, stop=sp)
                nc.tensor.matmul(a1h1, hi, at_sb[:, k, 512:1024], start=st, stop=sp)
            for k in range(KC):
                st, sp = (k == 0), (k == KC - 1)
                lo = H1b_sb[:, k, :]
                nc.tensor.matmul(a1l0, lo, at_sb[:, k, 0:512], start=st, stop=sp)
                nc.tensor.matmul(a1l1, lo, at_sb[:, k, 512:1024], start=st, stop=sp)
            # evacuate into per-encoder base-0 tiles (partition-shifted copies)
            a1_sb = [
                psm.tile([IN, NB], bf16, name=f"a1_sb{e}", tag=f"a1_sb{e}")
                for e in range(3)
            ]
            nc.vector.tensor_copy(out=a1_sb[0][:, 0:512], in_=a1h0[0:64, :])
            nc.vector.tensor_copy(out=a1_sb[0][:, 512:1024], in_=a1h1[0:64, :])
            nc.vector.tensor_copy(out=a1_sb[1][:, 0:512], in_=a1h0[64:128, :])
            nc.vector.tensor_copy(out=a1_sb[1][:, 512:1024], in_=a1h1[64:128, :])
            nc.vector.tensor_copy(out=a1_sb[2][:, 0:512], in_=a1l0)
            nc.vector.tensor_copy(out=a1_sb[2][:, 512:1024], in_=a1l1)

            def a1_enc(e):
                # feature-major agg1 slice for encoder e: [64, NB], base 0
                return a1_sb[e][:, :]

            # ---- stage E: cT (feature-major relu'd concat) + he (node-major)
            cT_hi = psm.tile([P, NB], bf16, name="cT_hi")
            cT_lo = psm.tile([F3 - P, NB], bf16, name="cT_lo")
            for e in range(3):
                for i in range(2):
                    ps = prot.tile([IN, 512], f32, name="ct_ps", tag="rot")
                    nc.tensor.matmul(
                        ps,
                        w2_sb[:, ds(e * HID, HID)],
                        a1_enc(e)[:, ds(i * 512, 512)],
                        start=True,
                        stop=True,
                    )
                    if e == 0:
                        dst = cT_hi[0:64, ds(i * 512, 512)]
                    elif e == 1:
                        dst = cT_hi[64:128, ds(i * 512, 512)]
                    else:
                        dst = cT_lo[0:64, ds(i * 512, 512)]
                    # partition-shifted relu evac (base 0 -> base 64 for e=1)
                    nc.vector.tensor_relu(out=dst, in_=ps)

            he_sb = psm.tile([P, 3, MT, IN], f32, name="he_sb")
            for e in range(3):
                for m in range(MT):
                    ps = prot.tile([P, IN], f32, name="he_ps", tag="rot")
                    nc.tensor.matmul(
                        ps,
                        a1_enc(e)[:, ds(m * P, P)],
                        w2_sb[:, ds(e * HID, HID)],
                        start=True,
                        stop=True,
                    )
                    nc.scalar.activation(he_sb[:, e, m, :], ps, Relu)

            # att_in = c @ att_W, node-major, evacuated to att_sb
            att_sb = psm.tile([P, MT, F3], f32, name="att_sb")
            for m in range(MT):
                ps = prot.tile([P, F3], f32, name="att_ps", tag="rot")
                nc.tensor.matmul(
                    ps, cT_hi[:, ds(m * P, P)], aw_hi, start=True, stop=False
                )
                nc.tensor.matmul(
                    ps, cT_lo[:, ds(m * P, P)], aw_lo, start=False, stop=True
                )
                nc.scalar.activation(att_sb[:, m, :], ps, Copy)

            # ---- stage F: softmax over j (groups of 3) in place, then fuse
            attv = att_sb.rearrange("p m (h j) -> p m j h", j=3)
            mx = psm.tile([P, MT, IN], f32, name="mx", tag="ftmp", bufs=3)
            nc.vector.tensor_max(out=mx, in0=attv[:, :, 0, :], in1=attv[:, :, 1, :])
            nc.vector.tensor_max(out=mx, in0=mx, in1=attv[:, :, 2, :])
            for j in range(3):
                nc.vector.tensor_sub(
                    out=attv[:, :, j, :], in0=attv[:, :, j, :], in1=mx
                )
            for j in range(3):
                nc.scalar.activation(attv[:, :, j, :], attv[:, :, j, :], Exp)
            ssum = psm.tile([P, MT, IN], f32, name="ssum", tag="ftmp", bufs=3)
            nc.vector.tensor_add(
                out=ssum, in0=attv[:, :, 0, :], in1=attv[:, :, 1, :]
            )
            nc.vector.tensor_add(out=ssum, in0=ssum, in1=attv[:, :, 2, :])
            rcp = psm.tile([P, MT, IN], f32, name="rcp", tag="ftmp", bufs=3)
            nc.vector.reciprocal_approx_fast(out=rcp, in_=ssum)
            for j in range(3):
                nc.vector.tensor_mul(
                    out=attv[:, :, j, :], in0=attv[:, :, j, :], in1=rcp
                )
            nc.sync.dma_start(
                out=att_rows.rearrange("(m p) f -> p m f", p=P), in_=att_sb
            )

            hacc = psm.tile([P, MT, IN], f32, name="hacc", tag="ftmp", bufs=3)
            htmp = psm.tile([P, MT, IN], f32, name="htmp", tag="ftmp", bufs=3)
            nc.vector.tensor_mul(out=hacc, in0=he_sb[:, 0], in1=attv[:, :, 0, :])
            nc.vector.tensor_mul(out=htmp, in0=he_sb[:, 1], in1=attv[:, :, 1, :])
            nc.vector.tensor_add(out=hacc, in0=hacc, in1=htmp)
            nc.vector.tensor_mul(out=htmp, in0=he_sb[:, 2], in1=attv[:, :, 2, :])
            h_sb = psm.tile([P, MT, IN], bf16, name="h_sb")
            nc.vector.tensor_add(out=h_sb, in0=hacc, in1=htmp)
            nc.sync.dma_start(out=ag2_in[:, :], in_=h_sb)

            # ---- AG2 + reload
            ag(ag2_in, ag2_out)
            H2_sb = ph.tile([P, KC, IN], bf16, name="H2_sb", tag="hbuf")
            nc.sync.dma_start(
                out=H2_sb, in_=ag2_out.rearrange("r p q -> p r q")
            )

            # ---- stage H: SpMM3: a2T = (A_hat h)^T
            a2_ps0 = pacc.tile([IN, 512], f32, name="a2_ps0", tag="accA")
            a2_ps1 = pacc.tile([IN, 512], f32, name="a2_ps1", tag="accB")
            for k in range(KC):
                st, sp = (k == 0), (k == KC - 1)
                nc.tensor.matmul(
                    a2_ps0, H2_sb[:, k, :], at_sb[:, k, 0:512], start=st, stop=sp
                )
                nc.tensor.matmul(
                    a2_ps1, H2_sb[:, k, :], at_sb[:, k, 512:1024], start=st, stop=sp
                )
            a2_sb = psm.tile([IN, NB], bf16, name="a2_sb")
            nc.vector.tensor_copy(out=a2_sb[:, 0:512], in_=a2_ps0)
            nc.vector.tensor_copy(out=a2_sb[:, 512:1024], in_=a2_ps1)

            # ---- stage I: decoder heads
            xo_sb = psm.tile([P, MT, IN], f32, name="xo_sb")
            for m in range(MT):
                ps = prot.tile([P, IN], f32, name="xo_ps", tag="rot")
                nc.tensor.matmul(
                    ps, a2_sb[:, ds(m * P, P)], dwx_sb, start=True, stop=True
                )
                nc.scalar.activation(xo_sb[:, m, :], ps, Copy)
            nc.sync.dma_start(
                out=x_rows.rearrange("(m p) f -> p m f", p=P), in_=xo_sb
            )

            hT_ps0 = pacc.tile([IN, 512], f32, name="hT_ps0", tag="accA")
            hT_ps1 = pacc.tile([IN, 512], f32, name="hT_ps1", tag="accB")
            nc.tensor.matmul(hT_ps0, dws_sb, a2_sb[:, 0:512], start=True, stop=True)
            nc.tensor.matmul(
                hT_ps1, dws_sb, a2_sb[:, 512:1024], start=True, stop=True
            )
            hT_sb = psm.tile([IN, NB], bf16, name="hT_sb")
            nc.vector.tensor_copy(out=hT_sb[:, 0:512], in_=hT_ps0)
            nc.vector.tensor_copy(out=hT_sb[:, 512:1024], in_=hT_ps1)
            nc.sync.dma_start(out=ag3_in[:, :], in_=hT_sb)

            # ---- AG3 + reload h_^T for all nodes
            ag(ag3_in, ag3_out)
            hTf_sb = ph.tile([IN, NCORES, NB], bf16, name="hTf_sb", tag="hbuf")
            nc.sync.dma_start(
                out=hTf_sb, in_=ag3_out.rearrange("(r f) m -> f r m", f=IN)
            )

            # ---- stage K: s_ rows = h_rows @ h_full^T (32 MB f32 out)
            dma_engines = [nc.sync, nc.scalar, nc.gpsimd]
            for m in range(MT):
                lhsT = hT_sb[:, ds(m * P, P)]
                for g in range(8):  # groups of 2 n-tiles -> one 512 KB DMA
                    ev = psm.tile([P, 1024], f32, name="s_ev", tag="sev", bufs=3)
                    for i in range(2):
                        n = g * 2 + i
                        ps = pacc.tile(
                            [P, 512], f32, name="s_ps",
                            tag=["accA", "accB", "accC", "accD"][n % 4],
                        )
                        nc.tensor.matmul(
                            ps,
                            lhsT,
                            hTf_sb[:, n // 2, ds((n % 2) * 512, 512)],
                            start=True,
                            stop=True,
                        )
                        if i % 2 == 0:
                            nc.scalar.copy(out=ev[:, ds(i * 512, 512)], in_=ps)
                        else:
                            nc.vector.tensor_copy(
                                out=ev[:, ds(i * 512, 512)], in_=ps
                            )
                    dma_engines[g % 3].dma_start(
                        out=s_rows[ds(m * P, P), ds(g * 1024, 1024)], in_=ev
                    )

    nc.finalize()
    return nc


def _get_program():
    global _PROG
    if _PROG is None:
        _PROG = _build_program()
    return _PROG


def kernel(**inputs) -> tuple:
    global LAST_EXEC_NS
    import ml_dtypes

    from concourse.bass_utils import run_bass_kernel_spmd

    bf = ml_dtypes.bfloat16

    x = np.asarray(inputs["x"], dtype=np.float32)
    src = np.asarray(inputs["src"]).astype(np.int64)
    dst = np.asarray(inputs["dst"]).astype(np.int64)

    # ---- host-side: bake the normalized adjacency (transposed), per hint:
    # edge partitioning by dst == column shards of A_hat^T.
    deg = 1.0 + np.bincount(dst, minlength=N).astype(np.float64)
    dinv = (1.0 / np.sqrt(deg)).astype(np.float32)
    coef = (dinv[src] * dinv[dst]).astype(np.float64)
    flat = np.bincount(src * N + dst, weights=coef, minlength=N * N)
    AT = flat.astype(np.float32).reshape(N, N)
    AT[np.arange(N), np.arange(N)] += dinv * dinv

    w1 = np.concatenate(
        [inputs["enc_a_W1"], inputs["enc_s_W1"], inputs["enc_t_W1"]], axis=1
    )
    w2 = np.concatenate(
        [inputs["enc_a_W2"], inputs["enc_s_W2"], inputs["enc_t_W2"]], axis=1
    )
    def to_sbuf_layout(mat):
        # [N, F] node-major -> [128, KC*F]: row p holds chunks k of node k*128+p
        f = mat.shape[1]
        return np.ascontiguousarray(
            mat.reshape(KC, P, f).transpose(1, 0, 2).reshape(P, KC * f)
        )

    common = {
        "xb": to_sbuf_layout(x.astype(bf)),
        "w1": np.ascontiguousarray(np.asarray(w1, np.float32).astype(bf)),
        "w2": np.ascontiguousarray(np.asarray(w2, np.float32).astype(bf)),
        "aw": np.ascontiguousarray(np.asarray(inputs["att_W"], np.float32).astype(bf)),
        "dwx": np.ascontiguousarray(
            np.asarray(inputs["dec_x_W"], np.float32).astype(bf)
        ),
        "dws": np.ascontiguousarray(
            np.asarray(inputs["dec_s_W"], np.float32).astype(bf)
        ),
    }
    in_maps = []
    for r in range(NCORES):
        m = dict(common)
        m["at"] = to_sbuf_layout(AT[:, r * NB : (r + 1) * NB].astype(bf))
        in_maps.append(m)

    nc = _get_program()
    kwargs = {}
    if TRACE:
        kwargs = dict(trace=True, trace_cores=list(range(NCORES)))
    res = run_bass_kernel_spmd(nc, in_maps, core_ids=list(range(NCORES)), **kwargs)
    LAST_EXEC_NS = res.exec_time_ns
    results = res.results

    s_ = np.concatenate([results[r]["s_rows"] for r in range(NCORES)], axis=0)
    x_ = np.concatenate([results[r]["x_rows"] for r in range(NCORES)], axis=0)
    att = np.concatenate(
        [results[r]["att_rows"] for r in range(NCORES)], axis=0
    ).reshape(N, HID, 3)
    return (
        np.asarray(x_, np.float32),
        np.asarray(s_, np.float32),
        np.asarray(att, np.float32),
    )


# revision 10
# speedup vs baseline: 1.5276x; 1.0940x over previous
"""Distributed Trainium2 kernel for the ADAGAD GNN message-passing model.

Model (see problem reference): three 2-layer GCN encoders over a shared
graph, attention-softmax fusion of the three embeddings, two GCN decoder
heads, and a final dense similarity matrix s_ = h_ @ h_.T.

Every GCN conv uses the same symmetric-normalized adjacency with self
loops, A_hat = D^-1/2 (A + I) D^-1/2 (D = 1 + in-degree).  The host
pre-bakes A_hat^T densely in bf16 and column-shards it over the 8 cores
(columns = destination nodes, matching the "partition edges by dst"
sharding).  Each core keeps its 16 MB shard resident in SBUF and runs all
sparse aggregations as dense TensorE matmuls in "outT" form:

    aggT[f, m] = sum_k H[k, f] * A_hatT[k, m]   (lhsT = H chunk, rhs = A_hatT)

which produces feature-major aggregates whose slices feed directly as
lhsT into the small dense-weight matmuls, flipping back to node-major
with no transposes anywhere.  Cross-core exchange is three bf16
AllGathers (H1 after encoder layer 1, h after fusion, h_^T before the
final row-sharded h_ @ h_.T whose 32 MB/core f32 output write is the
memory-roofline term).
"""

import numpy as np

N = 8192
IN = 64
HID = 64
F3 = 3 * HID          # 192
NCORES = 8
NB = N // NCORES      # 1024 rows (dst nodes) per core
P = 128               # partitions
KC = N // P           # 64 contraction chunks
MT = NB // P          # 8 m-tiles per core

TRACE = False         # set by test harness to collect HW exec time
LAST_EXEC_NS = None

_PROG = None


def _build_program():
    import concourse.bass as bass
    import concourse.mybir as mybir
    from concourse import bacc
    from concourse.bass import ds
    from concourse.tile import TileContext

    bf16 = mybir.dt.bfloat16
    f32 = mybir.dt.float32
    Relu = mybir.ActivationFunctionType.Relu
    Exp = mybir.ActivationFunctionType.Exp
    Copy = mybir.ActivationFunctionType.Copy
    RG = [list(range(NCORES))]

    nc = bacc.Bacc(None, num_devices=NCORES, target_bir_lowering=False, debug=True)

    # host pre-arranged to SBUF layout: [p, k, :] = row k*128+p of the
    # node-major matrix, flattened -> fully contiguous per-partition DMAs
    at = nc.declare_dram_parameter("at", [P, KC * NB], bf16, isOutput=False)
    xb = nc.declare_dram_parameter("xb", [P, KC * IN], bf16, isOutput=False)
    w1 = nc.declare_dram_parameter("w1", [IN, F3], bf16, isOutput=False)
    w2 = nc.declare_dram_parameter("w2", [HID, F3], bf16, isOutput=False)
    aw = nc.declare_dram_parameter("aw", [F3, F3], bf16, isOutput=False)
    dwx = nc.declare_dram_parameter("dwx", [HID, IN], bf16, isOutput=False)
    dws = nc.declare_dram_parameter("dws", [HID, IN], bf16, isOutput=False)

    s_rows = nc.declare_dram_parameter("s_rows", [NB, N], f32, isOutput=True)
    x_rows = nc.declare_dram_parameter("x_rows", [NB, IN], f32, isOutput=True)
    att_rows = nc.declare_dram_parameter("att_rows", [NB, F3], f32, isOutput=True)

    # AG1/AG2 bounces keep the SBUF tile layout: in = [P, MT*F], out adds a
    # leading rank dim; global chunk k = r*MT + m matches at_sb row order.
    ag1a_in = nc.dram_tensor("ag1a_in", [P, MT * P], bf16)
    ag1a_out = nc.dram_tensor("ag1a_out", [NCORES, P, MT * P], bf16, addr_space="Shared")
    ag1b_in = nc.dram_tensor("ag1b_in", [P, MT * (F3 - P)], bf16)
    ag1b_out = nc.dram_tensor("ag1b_out", [NCORES, P, MT * (F3 - P)], bf16, addr_space="Shared")
    ag2_in = nc.dram_tensor("ag2_in", [P, MT * HID], bf16)
    ag2_out = nc.dram_tensor("ag2_out", [NCORES, P, MT * HID], bf16, addr_space="Shared")
    agw_in = nc.dram_tensor("agw_in", [1, 16], bf16)
    agw_out = nc.dram_tensor("agw_out", [NCORES, 16], bf16, addr_space="Shared")
    ag3_in = nc.dram_tensor("ag3_in", [IN, NB], bf16)
    ag3_out = nc.dram_tensor("ag3_out", [NCORES * IN, NB], bf16, addr_space="Shared")

    def ag(in_t, out_t):
        nc.gpsimd.collective_compute(
            "AllGather",
            mybir.AluOpType.bypass,
            replica_groups=RG,
            ins=[in_t[...]],
            outs=[out_t[...]],
        )

    with TileContext(nc) as tc:
        with (
            tc.tile_pool(name="pat", bufs=1) as pat,
            tc.tile_pool(name="ph", bufs=1) as ph,
            tc.tile_pool(name="pw", bufs=1) as pw,
            tc.tile_pool(name="psm", bufs=1) as psm,
            tc.tile_pool(name="pacc", bufs=1, space="PSUM") as pacc,
            tc.tile_pool(name="prot", bufs=2, space="PSUM") as prot,
        ):
            # warm up the collective path while the big loads stream
            ag(agw_in, agw_out)

            # ---- weights
            w1_sb = pw.tile([IN, F3], bf16, name="w1_sb")
            nc.sync.dma_start(out=w1_sb, in_=w1[:, :])
            w2_sb = pw.tile([HID, F3], bf16, name="w2_sb")
            nc.sync.dma_start(out=w2_sb, in_=w2[:, :])
            aw_hi = pw.tile([P, F3], bf16, name="aw_hi")
            nc.sync.dma_start(out=aw_hi, in_=aw[0:P, :])
            aw_lo = pw.tile([F3 - P, F3], bf16, name="aw_lo")
            nc.sync.dma_start(out=aw_lo, in_=aw[P:F3, :])
            dwx_sb = pw.tile([HID, IN], bf16, name="dwx_sb")
            nc.sync.dma_start(out=dwx_sb, in_=dwx[:, :])
            dws_sb = pw.tile([HID, IN], bf16, name="dws_sb")
            nc.sync.dma_start(out=dws_sb, in_=dws[:, :])

            # ---- stage A: adjacency + x loads, SpMM1: a0T = (A_hat x)^T
            x_sb = ph.tile([P, KC, IN], bf16, name="x_sb", tag="hbuf")
            nc.scalar.dma_start(out=x_sb, in_=xb[:, :])
            at_sb = pat.tile([P, KC, NB], bf16, name="at_sb")
            for g in range(16):
                eng = nc.sync if g % 2 == 0 else nc.scalar
                eng.dma_start(
                    out=at_sb[:, g * 4 : (g + 1) * 4, :],
                    in_=at[:, g * 4 * NB : (g + 1) * 4 * NB],
                )

            a0_ps0 = pacc.tile([IN, 512], f32, name="a0_ps0", tag="accA")
            a0_ps1 = pacc.tile([IN, 512], f32, name="a0_ps1", tag="accB")
            for k in range(KC):
                st, sp = (k == 0), (k == KC - 1)
                nc.tensor.matmul(
                    a0_ps0, x_sb[:, k, :], at_sb[:, k, 0:512], start=st, stop=sp
                )
                nc.tensor.matmul(
                    a0_ps1, x_sb[:, k, :], at_sb[:, k, 512:1024], start=st, stop=sp
                )
            a0_sb = psm.tile([IN, NB], bf16, name="a0_sb")
            nc.vector.tensor_copy(out=a0_sb[:, 0:512], in_=a0_ps0)
            nc.vector.tensor_copy(out=a0_sb[:, 512:1024], in_=a0_ps1)

            # ---- stage B: H1 = relu(a0 @ W1cat), node-major, split hi/lo
            h1a_sb = psm.tile([P, MT, P], bf16, name="h1a_sb")
            h1b_sb = psm.tile([P, MT, F3 - P], bf16, name="h1b_sb")
            for m in range(MT):
                ps = prot.tile([P, F3], f32, name="h1_ps", tag="rot")
                nc.tensor.matmul(
                    ps, a0_sb[:, ds(m * P, P)], w1_sb, start=True, stop=True
                )
                nc.scalar.activation(h1a_sb[:, m, :], ps[:, 0:P], Relu)
                nc.scalar.activation(h1b_sb[:, m, :], ps[:, P:F3], Relu)
            nc.sync.dma_start(out=ag1a_in[:, :], in_=h1a_sb)
            nc.sync.dma_start(out=ag1b_in[:, :], in_=h1b_sb)

            # ---- AG1 (split: hi lands first, D-hi starts while lo flies)
            ag(ag1a_in, ag1a_out)
            ag(ag1b_in, ag1b_out)
            H1a_sb = ph.tile([P, KC, P], bf16, name="H1a_sb", tag="hbuf")
            nc.sync.dma_start(
                out=H1a_sb, in_=ag1a_out.rearrange("r p q -> p r q")
            )
            H1b_sb = ph.tile([P, KC, F3 - P], bf16, name="H1b_sb", tag="hbuf2")
            nc.scalar.dma_start(
                out=H1b_sb, in_=ag1b_out.rearrange("r p q -> p r q")
            )

            # ---- stage D: SpMM2: a1T = (A_hat H1)^T, two stationary pieces
            a1h0 = pacc.tile([P, 512], f32, name="a1h0", tag="accA")
            a1h1 = pacc.tile([P, 512], f32, name="a1h1", tag="accB")
            a1l0 = pacc.tile([F3 - P, 512], f32, name="a1l0", tag="accC")
            a1l1 = pacc.tile([F3 - P, 512], f32, name="a1l1", tag="accD")
            for k in range(KC):
                st, sp = (k == 0), (k == KC - 1)
                hi = H1a_sb[:, k, :]
                nc.tensor.matmul(a1h0, hi, at_sb[:, k, 0:512], start=st, stop=sp)
                nc.tensor.matmul(a1h1, hi, at_sb[:, k, 512:1024], start=st, stop=sp)
            for k in range(KC):
                st, sp = (k == 0), (k == KC - 1)
                lo = H1b_sb[:, k, :]
                nc.tensor.matmul(a1l0, lo, at_sb[:, k, 0:512], start=st, stop=sp)
                nc.tensor.matmul(a1l1, lo, at_sb[:, k, 512:1024], start=st, stop=sp)
            # evacuate into per-encoder base-0 tiles (partition-shifted copies)
            a1_sb = [
                psm.tile([IN, NB], bf16, name=f"a1_sb{e}", tag=f"a1_sb{e}")
                for e in range(3)
            ]
            nc.vector.tensor_copy(out=a1_sb[0][:, 0:512], in_=a1h0[0:64, :])
            nc.vector.tensor_copy(out=a1_sb[0][:, 512:1024], in_=a1h1[0:64, :])
            nc.vector.tensor_copy(out=a1_sb[1][:, 0:512], in_=a1h0[64:128, :])
            nc.vector.tensor_copy(out=a1_sb[1][:, 512:1024], in_=a1h1[64:128, :])
            nc.vector.tensor_copy(out=a1_sb[2][:, 0:512], in_=a1l0)
            nc.vector.tensor_copy(out=a1_sb[2][:, 512:1024], in_=a1l1)

            def a1_enc(e):
                # feature-major agg1 slice for encoder e: [64, NB], base 0
                return a1_sb[e][:, :]

            # ---- stage E: cT (feature-major relu'd concat) + he (node-major)
            cT_hi = psm.tile([P, NB], bf16, name="cT_hi")
            cT_lo = psm.tile([F3 - P, NB], bf16, name="cT_lo")
            for e in range(3):
                for i in range(2):
                    ps = prot.tile([IN, 512], f32, name="ct_ps", tag="rot")
                    nc.tensor.matmul(
                        ps,
                        w2_sb[:, ds(e * HID, HID)],
                        a1_enc(e)[:, ds(i * 512, 512)],
                        start=True,
                        stop=True,
                    )
                    if e == 0:
                        dst = cT_hi[0:64, ds(i * 512, 512)]
                    elif e == 1:
                        dst = cT_hi[64:128, ds(i * 512, 512)]
                    else:
                        dst = cT_lo[0:64, ds(i * 512, 512)]
                    # partition-shifted relu evac (base 0 -> base 64 for e=1)
                    nc.vector.tensor_relu(out=dst, in_=ps)

            he_sb = psm.tile([P, 3, MT, IN], f32, name="he_sb")
            for e in range(3):
                for m in range(MT):
                    ps = prot.tile([P, IN], f32, name="he_ps", tag="rot")
                    nc.tensor.matmul(
                        ps,
                        a1_enc(e)[:, ds(m * P, P)],
                        w2_sb[:, ds(e * HID, HID)],
                        start=True,
                        stop=True,
                    )
                    nc.scalar.activation(he_sb[:, e, m, :], ps, Relu)

            # att_in = c @ att_W, node-major, evacuated to att_sb
            att_sb = psm.tile([P, MT, F3], f32, name="att_sb")
            for m in range(MT):
                ps = prot.tile([P, F3], f32, name="att_ps", tag="rot")
                nc.tensor.matmul(
                    ps, cT_hi[:, ds(m * P, P)], aw_hi, start=True, stop=False
                )
                nc.tensor.matmul(
                    ps, cT_lo[:, ds(m * P, P)], aw_lo, start=False, stop=True
                )
                nc.scalar.activation(att_sb[:, m, :], ps, Copy)

            # ---- stage F: softmax over j (groups of 3) in place, then fuse
            attv = att_sb.rearrange("p m (h j) -> p m j h", j=3)
            mx = psm.tile([P, MT, IN], f32, name="mx", tag="ftmp", bufs=2)
            nc.vector.tensor_max(out=mx, in0=attv[:, :, 0, :], in1=attv[:, :, 1, :])
            nc.vector.tensor_max(out=mx, in0=mx, in1=attv[:, :, 2, :])
            for j in range(3):
                nc.vector.tensor_sub(
                    out=attv[:, :, j, :], in0=attv[:, :, j, :], in1=mx
                )
            for j in range(3):
                nc.scalar.activation(attv[:, :, j, :], attv[:, :, j, :], Exp)
            ssum = psm.tile([P, MT, IN], f32, name="ssum", tag="ftmp", bufs=2)
            nc.vector.tensor_add(
                out=ssum, in0=attv[:, :, 0, :], in1=attv[:, :, 1, :]
            )
            nc.vector.tensor_add(out=ssum, in0=ssum, in1=attv[:, :, 2, :])
            rcp = psm.tile([P, MT, IN], f32, name="rcp", tag="ftmp", bufs=2)
            nc.vector.reciprocal_approx_fast(out=rcp, in_=ssum)
            for j in range(3):
                nc.vector.tensor_mul(
                    out=attv[:, :, j, :], in0=attv[:, :, j, :], in1=rcp
                )
            nc.sync.dma_start(
                out=att_rows.rearrange("(m p) f -> p m f", p=P), in_=att_sb
            )

            hacc = psm.tile([P, MT, IN], f32, name="hacc", tag="ftmp", bufs=2)
            htmp = psm.tile([P, MT, IN], f32, name="htmp", tag="ftmp", bufs=2)
            nc.vector.tensor_mul(out=hacc, in0=he_sb[:, 0], in1=attv[:, :, 0, :])
            nc.vector.tensor_mul(out=htmp, in0=he_sb[:, 1], in1=attv[:, :, 1, :])
            nc.vector.tensor_add(out=hacc, in0=hacc, in1=htmp)
            nc.vector.tensor_mul(out=htmp, in0=he_sb[:, 2], in1=attv[:, :, 2, :])
            h_sb = psm.tile([P, MT, IN], bf16, name="h_sb")
            nc.vector.tensor_add(out=h_sb, in0=hacc, in1=htmp)
            nc.sync.dma_start(out=ag2_in[:, :], in_=h_sb)

            # ---- AG2 + reload
            ag(ag2_in, ag2_out)
            H2_sb = ph.tile([P, KC, IN], bf16, name="H2_sb", tag="hbuf")
            nc.sync.dma_start(
                out=H2_sb, in_=ag2_out.rearrange("r p q -> p r q")
            )

            # ---- stage H: SpMM3: a2T = (A_hat h)^T
            a2_ps0 = pacc.tile([IN, 512], f32, name="a2_ps0", tag="accA")
            a2_ps1 = pacc.tile([IN, 512], f32, name="a2_ps1", tag="accB")
            for k in range(KC):
                st, sp = (k == 0), (k == KC - 1)
                nc.tensor.matmul(
                    a2_ps0, H2_sb[:, k, :], at_sb[:, k, 0:512], start=st, stop=sp
                )
                nc.tensor.matmul(
                    a2_ps1, H2_sb[:, k, :], at_sb[:, k, 512:1024], start=st, stop=sp
                )
            a2_sb = psm.tile([IN, NB], bf16, name="a2_sb")
            nc.vector.tensor_copy(out=a2_sb[:, 0:512], in_=a2_ps0)
            nc.vector.tensor_copy(out=a2_sb[:, 512:1024], in_=a2_ps1)

            # ---- stage I: decoder heads
            xo_sb = psm.tile([P, MT, IN], f32, name="xo_sb")
            for m in range(MT):
                ps = prot.tile([P, IN], f32, name="xo_ps", tag="rot")
                nc.tensor.matmul(
                    ps, a2_sb[:, ds(m * P, P)], dwx_sb, start=True, stop=True
                )
                nc.scalar.activation(xo_sb[:, m, :], ps, Copy)
            nc.sync.dma_start(
                out=x_rows.rearrange("(m p) f -> p m f", p=P), in_=xo_sb
            )

            hT_ps0 = pacc.tile([IN, 512], f32, name="hT_ps0", tag="accA")
            hT_ps1 = pacc.tile([IN, 512], f32, name="hT_ps1", tag="accB")
            nc.tensor.matmul(hT_ps0, dws_sb, a2_sb[:, 0:512], start=True, stop=True)
            nc.tensor.matmul(
                hT_ps1, dws_sb, a2_sb[:, 512:1024], start=True, stop=True
            )
            hT_sb = psm.tile([IN, NB], bf16, name="hT_sb")
            nc.vector.tensor_copy(out=hT_sb[:, 0:512], in_=hT_ps0)
            nc.vector.tensor_copy(out=hT_sb[:, 512:1024], in_=hT_ps1)
            nc.sync.dma_start(out=ag3_in[:, :], in_=hT_sb)

            # ---- AG3 + reload h_^T for all nodes
            ag(ag3_in, ag3_out)
            hTf_sb = ph.tile([IN, NCORES, NB], bf16, name="hTf_sb", tag="hbuf")
            nc.sync.dma_start(
                out=hTf_sb, in_=ag3_out.rearrange("(r f) m -> f r m", f=IN)
            )

            # ---- stage K: s_ rows = h_rows @ h_full^T (32 MB f32 out)
            dma_engines = [nc.sync, nc.scalar, nc.gpsimd]
            for m in range(MT):
                lhsT = hT_sb[:, ds(m * P, P)]
                for g in range(8):  # groups of 2 n-tiles -> one 512 KB DMA
                    ev = psm.tile([P, 1024], f32, name="s_ev", tag="sev", bufs=4)
                    for i in range(2):
                        n = g * 2 + i
                        ps = pacc.tile(
                            [P, 512], f32, name="s_ps",
                            tag=["accA", "accB", "accC", "accD"][n % 4],
                        )
                        nc.tensor.matmul(
                            ps,
                            lhsT,
                            hTf_sb[:, n // 2, ds((n % 2) * 512, 512)],
                            start=True,
                            stop=True,
                        )
                        if i % 2 == 0:
                            nc.scalar.copy(out=ev[:, ds(i * 512, 512)], in_=ps)
                        else:
                            nc.vector.tensor_copy(
                                out=ev[:, ds(i * 512, 512)], in_=ps
                            )
                    dma_engines[g % 3].dma_start(
                        out=s_rows[ds(m * P, P), ds(g * 1024, 1024)], in_=ev
                    )

    nc.finalize()
    return nc


def _get_program():
    global _PROG
    if _PROG is None:
        _PROG = _build_program()
    return _PROG


def kernel(**inputs) -> tuple:
    global LAST_EXEC_NS
    import ml_dtypes

    from concourse.bass_utils import run_bass_kernel_spmd

    bf = ml_dtypes.bfloat16

    x = np.asarray(inputs["x"], dtype=np.float32)
    src = np.asarray(inputs["src"]).astype(np.int64)
    dst = np.asarray(inputs["dst"]).astype(np.int64)

    # ---- host-side: bake the normalized adjacency (transposed), per hint:
    # edge partitioning by dst == column shards of A_hat^T.
    deg = 1.0 + np.bincount(dst, minlength=N).astype(np.float64)
    dinv = (1.0 / np.sqrt(deg)).astype(np.float32)
    coef = (dinv[src] * dinv[dst]).astype(np.float64)
    flat = np.bincount(src * N + dst, weights=coef, minlength=N * N)
    AT = flat.astype(np.float32).reshape(N, N)
    AT[np.arange(N), np.arange(N)] += dinv * dinv

    w1 = np.concatenate(
        [inputs["enc_a_W1"], inputs["enc_s_W1"], inputs["enc_t_W1"]], axis=1
    )
    w2 = np.concatenate(
        [inputs["enc_a_W2"], inputs["enc_s_W2"], inputs["enc_t_W2"]], axis=1
    )
    def to_sbuf_layout(mat):
        # [N, F] node-major -> [128, KC*F]: row p holds chunks k of node k*128+p
        f = mat.shape[1]
        return np.ascontiguousarray(
            mat.reshape(KC, P, f).transpose(1, 0, 2).reshape(P, KC * f)
        )

    common = {
        "xb": to_sbuf_layout(x.astype(bf)),
        "w1": np.ascontiguousarray(np.asarray(w1, np.float32).astype(bf)),
        "w2": np.ascontiguousarray(np.asarray(w2, np.float32).astype(bf)),
        "aw": np.ascontiguousarray(np.asarray(inputs["att_W"], np.float32).astype(bf)),
        "dwx": np.ascontiguousarray(
            np.asarray(inputs["dec_x_W"], np.float32).astype(bf)
        ),
        "dws": np.ascontiguousarray(
            np.asarray(inputs["dec_s_W"], np.float32).astype(bf)
        ),
    }
    in_maps = []
    for r in range(NCORES):
        m = dict(common)
        m["at"] = to_sbuf_layout(AT[:, r * NB : (r + 1) * NB].astype(bf))
        in_maps.append(m)

    nc = _get_program()
    kwargs = {}
    if TRACE:
        kwargs = dict(trace=True, trace_cores=list(range(NCORES)))
    res = run_bass_kernel_spmd(nc, in_maps, core_ids=list(range(NCORES)), **kwargs)
    LAST_EXEC_NS = res.exec_time_ns
    results = res.results

    s_ = np.concatenate([results[r]["s_rows"] for r in range(NCORES)], axis=0)
    x_ = np.concatenate([results[r]["x_rows"] for r in range(NCORES)], axis=0)
    att = np.concatenate(
        [results[r]["att_rows"] for r in range(NCORES)], axis=0
    ).reshape(N, HID, 3)
    return (
        np.asarray(x_, np.float32),
        np.asarray(s_, np.float32),
        np.asarray(att, np.float32),
    )


# revision 12
# speedup vs baseline: 1.5614x; 1.0221x over previous
"""Distributed Trainium2 kernel for the ADAGAD GNN message-passing model.

Model (see problem reference): three 2-layer GCN encoders over a shared
graph, attention-softmax fusion of the three embeddings, two GCN decoder
heads, and a final dense similarity matrix s_ = h_ @ h_.T.

Every GCN conv uses the same symmetric-normalized adjacency with self
loops, A_hat = D^-1/2 (A + I) D^-1/2 (D = 1 + in-degree).  The host
pre-bakes A_hat^T densely in bf16 and column-shards it over the 8 cores
(columns = destination nodes, matching the "partition edges by dst"
sharding).  Each core keeps its 16 MB shard resident in SBUF and runs all
sparse aggregations as dense TensorE matmuls in "outT" form:

    aggT[f, m] = sum_k H[k, f] * A_hatT[k, m]   (lhsT = H chunk, rhs = A_hatT)

which produces feature-major aggregates whose slices feed directly as
lhsT into the small dense-weight matmuls, flipping back to node-major
with no transposes anywhere.  Cross-core exchange is three bf16
AllGathers (H1 after encoder layer 1, h after fusion, h_^T before the
final row-sharded h_ @ h_.T whose 32 MB/core f32 output write is the
memory-roofline term).
"""

import numpy as np

N = 8192
IN = 64
HID = 64
F3 = 3 * HID          # 192
NCORES = 8
NB = N // NCORES      # 1024 rows (dst nodes) per core
P = 128               # partitions
KC = N // P           # 64 contraction chunks
MT = NB // P          # 8 m-tiles per core

TRACE = False         # set by test harness to collect HW exec time
LAST_EXEC_NS = None

_PROG = None


def _build_program():
    import concourse.bass as bass
    import concourse.mybir as mybir
    from concourse import bacc
    from concourse.bass import ds
    from concourse.tile import TileContext

    bf16 = mybir.dt.bfloat16
    f32 = mybir.dt.float32
    f8 = mybir.dt.float8e4
    f8 = mybir.dt.float8e4
    Relu = mybir.ActivationFunctionType.Relu
    Exp = mybir.ActivationFunctionType.Exp
    Copy = mybir.ActivationFunctionType.Copy
    RG = [list(range(NCORES))]

    nc = bacc.Bacc(None, num_devices=NCORES, target_bir_lowering=False, debug=True)

    # host pre-arranged to SBUF layout: [p, k, :] = row k*128+p of the
    # node-major matrix, flattened -> fully contiguous per-partition DMAs
    # A+I edge counts, exact small integers in fp8; D^-1/2 folded into evacs
    at = nc.declare_dram_parameter("at", [P, KC * NB], f8, isOutput=False)
    xb = nc.declare_dram_parameter("xb", [P, KC * IN], bf16, isOutput=False)
    dv = nc.declare_dram_parameter("dv", [P, MT], f32, isOutput=False)
    dv2 = nc.declare_dram_parameter("dv2", [P, MT], f32, isOutput=False)
    dvr = nc.declare_dram_parameter("dvr", [IN, NB], f32, isOutput=False)
    w1 = nc.declare_dram_parameter("w1", [IN, F3], bf16, isOutput=False)
    w2 = nc.declare_dram_parameter("w2", [HID, F3], bf16, isOutput=False)
    aw = nc.declare_dram_parameter("aw", [F3, F3], bf16, isOutput=False)
    dwx = nc.declare_dram_parameter("dwx", [HID, IN], bf16, isOutput=False)
    dws = nc.declare_dram_parameter("dws", [HID, IN], bf16, isOutput=False)

    s_rows = nc.declare_dram_parameter("s_rows", [NB, N], f32, isOutput=True)
    x_rows = nc.declare_dram_parameter("x_rows", [NB, IN], f32, isOutput=True)
    att_rows = nc.declare_dram_parameter("att_rows", [NB, F3], f32, isOutput=True)

    # AG1/AG2 bounces keep the SBUF tile layout: in = [P, MT*F], out adds a
    # leading rank dim; global chunk k = r*MT + m matches at_sb row order.
    ag1_in = nc.dram_tensor("ag1_in", [P, MT * F3], bf16)
    ag1_out = nc.dram_tensor("ag1_out", [NCORES, P, MT * F3], bf16, addr_space="Shared")
    ag2_in = nc.dram_tensor("ag2_in", [P, MT * HID], bf16)
    ag2_out = nc.dram_tensor("ag2_out", [NCORES, P, MT * HID], bf16, addr_space="Shared")
    agw_in = nc.dram_tensor("agw_in", [1, 16], bf16)
    agw_out = nc.dram_tensor("agw_out", [NCORES, 16], bf16, addr_space="Shared")
    ag3_in = nc.dram_tensor("ag3_in", [IN, NB], bf16)
    ag3_out = nc.dram_tensor("ag3_out", [NCORES * IN, NB], bf16, addr_space="Shared")

    def ag(in_t, out_t):
        nc.gpsimd.collective_compute(
            "AllGather",
            mybir.AluOpType.bypass,
            replica_groups=RG,
            ins=[in_t[...]],
            outs=[out_t[...]],
        )

    with TileContext(nc) as tc:
        with (
            tc.tile_pool(name="pat", bufs=1) as pat,
            tc.tile_pool(name="ph", bufs=1) as ph,
            tc.tile_pool(name="pw", bufs=1) as pw,
            tc.tile_pool(name="psm", bufs=1) as psm,
            tc.tile_pool(name="pacc", bufs=1, space="PSUM") as pacc,
            tc.tile_pool(name="prot", bufs=2, space="PSUM") as prot,
        ):
            # warm up the collective path while the big loads stream
            ag(agw_in, agw_out)

            # ---- weights
            w1_sb = pw.tile([IN, F3], bf16, name="w1_sb")
            nc.sync.dma_start(out=w1_sb, in_=w1[:, :])
            w2_sb = pw.tile([HID, F3], bf16, name="w2_sb")
            nc.sync.dma_start(out=w2_sb, in_=w2[:, :])
            aw_hi = pw.tile([P, F3], bf16, name="aw_hi")
            nc.sync.dma_start(out=aw_hi, in_=aw[0:P, :])
            aw_lo = pw.tile([F3 - P, F3], bf16, name="aw_lo")
            nc.sync.dma_start(out=aw_lo, in_=aw[P:F3, :])
            dwx_sb = pw.tile([HID, IN], bf16, name="dwx_sb")
            nc.sync.dma_start(out=dwx_sb, in_=dwx[:, :])
            dws_sb = pw.tile([HID, IN], bf16, name="dws_sb")
            nc.sync.dma_start(out=dws_sb, in_=dws[:, :])

            # ---- stage A: adjacency + x loads, SpMM1: a0T = (A_hat x)^T
            x_sb = ph.tile([P, KC, IN], bf16, name="x_sb", tag="hbuf")
            nc.scalar.dma_start(out=x_sb, in_=xb[:, :])
            dv_sb = pw.tile([P, MT], f32, name="dv_sb")
            nc.scalar.dma_start(out=dv_sb, in_=dv[:, :])
            dv2_sb = pw.tile([P, MT], f32, name="dv2_sb")
            nc.scalar.dma_start(out=dv2_sb, in_=dv2[:, :])
            dvr_sb = pw.tile([IN, NB], f32, name="dvr_sb")
            nc.scalar.dma_start(out=dvr_sb, in_=dvr[:, :])
            at_sb = pat.tile([P, KC, NB], f8, name="at_sb")
            for g in range(8):
                eng = nc.sync if g % 2 == 0 else nc.scalar
                eng.dma_start(
                    out=at_sb[:, g * 8 : (g + 1) * 8, :],
                    in_=at[:, g * 8 * NB : (g + 1) * 8 * NB],
                )

            a0_ps0 = pacc.tile([IN, 512], f32, name="a0_ps0", tag="accA")
            a0_ps1 = pacc.tile([IN, 512], f32, name="a0_ps1", tag="accB")
            for k in range(KC):
                st, sp = (k == 0), (k == KC - 1)
                nc.tensor.matmul(
                    a0_ps0, x_sb[:, k, :], at_sb[:, k, 0:512], start=st, stop=sp
                )
                nc.tensor.matmul(
                    a0_ps1, x_sb[:, k, :], at_sb[:, k, 512:1024], start=st, stop=sp
                )
            a0_sb = psm.tile([IN, NB], bf16, name="a0_sb")
            nc.vector.tensor_copy(out=a0_sb[:, 0:512], in_=a0_ps0)
            nc.vector.tensor_copy(out=a0_sb[:, 512:1024], in_=a0_ps1)

            # ---- stage B: u1 = dinv*relu(dinv*(a0 @ W1)) = relu(dinv^2 * .)
            h1_sb = psm.tile([P, MT, F3], bf16, name="h1_sb")
            for m in range(MT):
                ps = prot.tile([P, F3], f32, name="h1_ps", tag="rot")
                nc.tensor.matmul(
                    ps, a0_sb[:, ds(m * P, P)], w1_sb, start=True, stop=True
                )
                nc.scalar.activation(
                    h1_sb[:, m, :], ps, Relu, scale=dv2_sb[:, m : m + 1]
                )
            nc.sync.dma_start(out=ag1_in[:, :], in_=h1_sb)

            # ---- AG1
            ag(ag1_in, ag1_out)
            H1_sb = ph.tile([P, KC, F3], bf16, name="H1_sb", tag="hbuf")
            nc.sync.dma_start(
                out=H1_sb, in_=ag1_out.rearrange("r p q -> p r q")
            )

            # ---- stage D: SpMM2: a1T = (A_hat H1)^T, two stationary pieces
            a1h0 = pacc.tile([P, 512], f32, name="a1h0", tag="accA")
            a1h1 = pacc.tile([P, 512], f32, name="a1h1", tag="accB")
            a1l0 = pacc.tile([F3 - P, 512], f32, name="a1l0", tag="accC")
            a1l1 = pacc.tile([F3 - P, 512], f32, name="a1l1", tag="accD")
            for k in range(KC):
                st, sp = (k == 0), (k == KC - 1)
                hi = H1_sb[:, k, 0:P]
                nc.tensor.matmul(a1h0, hi, at_sb[:, k, 0:512], start=st, stop=sp)
                nc.tensor.matmul(a1h1, hi, at_sb[:, k, 512:1024], start=st, stop=sp)
            for k in range(KC):
                st, sp = (k == 0), (k == KC - 1)
                lo = H1_sb[:, k, P:F3]
                nc.tensor.matmul(a1l0, lo, at_sb[:, k, 0:512], start=st, stop=sp)
                nc.tensor.matmul(a1l1, lo, at_sb[:, k, 512:1024], start=st, stop=sp)
            # evacuate into per-encoder base-0 tiles (partition-shifted copies)
            a1_sb = [
                psm.tile([IN, NB], bf16, name=f"a1_sb{e}", tag=f"a1_sb{e}")
                for e in range(3)
            ]
            nc.vector.tensor_copy(out=a1_sb[0][:, 0:512], in_=a1h0[0:64, :])
            nc.vector.tensor_copy(out=a1_sb[0][:, 512:1024], in_=a1h1[0:64, :])
            nc.vector.tensor_copy(out=a1_sb[1][:, 0:512], in_=a1h0[64:128, :])
            nc.vector.tensor_copy(out=a1_sb[1][:, 512:1024], in_=a1h1[64:128, :])
            nc.vector.tensor_copy(out=a1_sb[2][:, 0:512], in_=a1l0)
            nc.vector.tensor_copy(out=a1_sb[2][:, 512:1024], in_=a1l1)

            def a1_enc(e):
                # feature-major agg1 slice for encoder e: [64, NB], base 0
                return a1_sb[e][:, :]

            # ---- stage E: cT (feature-major relu'd concat) + he (node-major)
            cT_hi = psm.tile([P, NB], bf16, name="cT_hi")
            cT_lo = psm.tile([F3 - P, NB], bf16, name="cT_lo")
            for e in range(3):
                for i in range(2):
                    ps = prot.tile([IN, 512], f32, name="ct_ps", tag="rot")
                    nc.tensor.matmul(
                        ps,
                        w2_sb[:, ds(e * HID, HID)],
                        a1_enc(e)[:, ds(i * 512, 512)],
                        start=True,
                        stop=True,
                    )
                    if e == 0:
                        dst = cT_hi[0:64, ds(i * 512, 512)]
                    elif e == 1:
                        dst = cT_hi[64:128, ds(i * 512, 512)]
                    else:
                        dst = cT_lo[0:64, ds(i * 512, 512)]
                    # partition-shifted relu evac (base 0 -> base 64 for e=1)
                    nc.vector.tensor_relu(out=dst, in_=ps)

            he_sb = psm.tile([P, 3, MT, IN], f32, name="he_sb")
            for e in range(3):
                for m in range(MT):
                    ps = prot.tile([P, IN], f32, name="he_ps", tag="rot")
                    nc.tensor.matmul(
                        ps,
                        a1_enc(e)[:, ds(m * P, P)],
                        w2_sb[:, ds(e * HID, HID)],
                        start=True,
                        stop=True,
                    )
                    nc.scalar.activation(
                        he_sb[:, e, m, :], ps, Relu, scale=dv_sb[:, m : m + 1]
                    )

            # att_in = c @ att_W, node-major, evacuated to att_sb
            att_sb = psm.tile([P, MT, F3], f32, name="att_sb")
            for m in range(MT):
                ps = prot.tile([P, F3], f32, name="att_ps", tag="rot")
                nc.tensor.matmul(
                    ps, cT_hi[:, ds(m * P, P)], aw_hi, start=True, stop=False
                )
                nc.tensor.matmul(
                    ps, cT_lo[:, ds(m * P, P)], aw_lo, start=False, stop=True
                )
                nc.scalar.activation(
                    att_sb[:, m, :], ps, Copy, scale=dv_sb[:, m : m + 1]
                )

            # ---- stage F: softmax over j (groups of 3) in place, then fuse
            attv = att_sb.rearrange("p m (h j) -> p m j h", j=3)
            mx = psm.tile([P, MT, IN], f32, name="mx", tag="ftmp", bufs=2)
            nc.vector.tensor_max(out=mx, in0=attv[:, :, 0, :], in1=attv[:, :, 1, :])
            nc.vector.tensor_max(out=mx, in0=mx, in1=attv[:, :, 2, :])
            for j in range(3):
                nc.vector.tensor_sub(
                    out=attv[:, :, j, :], in0=attv[:, :, j, :], in1=mx
                )
            for j in range(3):
                nc.scalar.activation(attv[:, :, j, :], attv[:, :, j, :], Exp)
            ssum = psm.tile([P, MT, IN], f32, name="ssum", tag="ftmp", bufs=2)
            nc.vector.tensor_add(
                out=ssum, in0=attv[:, :, 0, :], in1=attv[:, :, 1, :]
            )
            nc.vector.tensor_add(out=ssum, in0=ssum, in1=attv[:, :, 2, :])
            rcp = psm.tile([P, MT, IN], f32, name="rcp", tag="ftmp", bufs=2)
            nc.vector.reciprocal_approx_fast(out=rcp, in_=ssum)
            for j in range(3):
                nc.vector.tensor_mul(
                    out=attv[:, :, j, :], in0=attv[:, :, j, :], in1=rcp
                )
            nc.sync.dma_start(
                out=att_rows.rearrange("(m p) f -> p m f", p=P), in_=att_sb
            )

            hacc = psm.tile([P, MT, IN], f32, name="hacc", tag="ftmp", bufs=2)
            htmp = psm.tile([P, MT, IN], f32, name="htmp", tag="ftmp", bufs=2)
            nc.vector.tensor_mul(out=hacc, in0=he_sb[:, 0], in1=attv[:, :, 0, :])
            nc.vector.tensor_mul(out=htmp, in0=he_sb[:, 1], in1=attv[:, :, 1, :])
            nc.vector.tensor_add(out=hacc, in0=hacc, in1=htmp)
            nc.vector.tensor_mul(out=htmp, in0=he_sb[:, 2], in1=attv[:, :, 2, :])
            nc.vector.tensor_add(out=hacc, in0=hacc, in1=htmp)
            h_sb = psm.tile([P, MT, IN], bf16, name="h_sb")
            for m in range(MT):
                nc.vector.tensor_scalar_mul(
                    h_sb[:, m, :], hacc[:, m, :], dv_sb[:, m : m + 1]
                )
            nc.sync.dma_start(out=ag2_in[:, :], in_=h_sb)

            # ---- AG2 + reload
            ag(ag2_in, ag2_out)
            H2_sb = ph.tile([P, KC, IN], bf16, name="H2_sb", tag="hbuf")
            nc.sync.dma_start(
                out=H2_sb, in_=ag2_out.rearrange("r p q -> p r q")
            )

            # ---- stage H: SpMM3: a2T = (A_hat h)^T
            a2_ps0 = pacc.tile([IN, 512], f32, name="a2_ps0", tag="accA")
            a2_ps1 = pacc.tile([IN, 512], f32, name="a2_ps1", tag="accB")
            for k in range(KC):
                st, sp = (k == 0), (k == KC - 1)
                nc.tensor.matmul(
                    a2_ps0, H2_sb[:, k, :], at_sb[:, k, 0:512], start=st, stop=sp
                )
                nc.tensor.matmul(
                    a2_ps1, H2_sb[:, k, :], at_sb[:, k, 512:1024], start=st, stop=sp
                )
            a2_sb = psm.tile([IN, NB], bf16, name="a2_sb")
            nc.vector.tensor_copy(out=a2_sb[:, 0:512], in_=a2_ps0)
            nc.vector.tensor_copy(out=a2_sb[:, 512:1024], in_=a2_ps1)

            # ---- stage I: decoder heads
            xo_sb = psm.tile([P, MT, IN], f32, name="xo_sb")
            for m in range(MT):
                ps = prot.tile([P, IN], f32, name="xo_ps", tag="rot")
                nc.tensor.matmul(
                    ps, a2_sb[:, ds(m * P, P)], dwx_sb, start=True, stop=True
                )
                nc.scalar.activation(
                    xo_sb[:, m, :], ps, Copy, scale=dv_sb[:, m : m + 1]
                )
            nc.sync.dma_start(
                out=x_rows.rearrange("(m p) f -> p m f", p=P), in_=xo_sb
            )

            hT_ps0 = pacc.tile([IN, 512], f32, name="hT_ps0", tag="accA")
            hT_ps1 = pacc.tile([IN, 512], f32, name="hT_ps1", tag="accB")
            nc.tensor.matmul(hT_ps0, dws_sb, a2_sb[:, 0:512], start=True, stop=True)
            nc.tensor.matmul(
                hT_ps1, dws_sb, a2_sb[:, 512:1024], start=True, stop=True
            )
            hT_sb = psm.tile([IN, NB], bf16, name="hT_sb")
            nc.vector.tensor_mul(
                out=hT_sb[:, 0:512], in0=hT_ps0, in1=dvr_sb[:, 0:512]
            )
            nc.vector.tensor_mul(
                out=hT_sb[:, 512:1024], in0=hT_ps1, in1=dvr_sb[:, 512:1024]
            )
            nc.sync.dma_start(out=ag3_in[:, :], in_=hT_sb)

            # ---- AG3 + reload h_^T for all nodes
            ag(ag3_in, ag3_out)
            hTf_sb = ph.tile([IN, NCORES, NB], bf16, name="hTf_sb", tag="hbuf")
            nc.sync.dma_start(
                out=hTf_sb, in_=ag3_out.rearrange("(r f) m -> f r m", f=IN)
            )

            # ---- stage K: s_ rows = h_rows @ h_full^T (32 MB f32 out)
            dma_engines = [nc.sync, nc.scalar, nc.gpsimd]
            for m in range(MT):
                lhsT = hT_sb[:, ds(m * P, P)]
                for g in range(8):  # groups of 2 n-tiles -> one 512 KB DMA
                    ev = psm.tile([P, 1024], f32, name="s_ev", tag="sev", bufs=4)
                    for i in range(2):
                        n = g * 2 + i
                        ps = pacc.tile(
                            [P, 512], f32, name="s_ps",
                            tag=["accA", "accB", "accC", "accD"][n % 4],
                        )
                        nc.tensor.matmul(
                            ps,
                            lhsT,
                            hTf_sb[:, n // 2, ds((n % 2) * 512, 512)],
                            start=True,
                            stop=True,
                        )
                        if i % 2 == 0:
                            nc.scalar.copy(out=ev[:, ds(i * 512, 512)], in_=ps)
                        else:
                            nc.vector.tensor_copy(
                                out=ev[:, ds(i * 512, 512)], in_=ps
                            )
                    dma_engines[g % 3].dma_start(
                        out=s_rows[ds(m * P, P), ds(g * 1024, 1024)], in_=ev
                    )

    nc.finalize()
    return nc


def _get_program():
    global _PROG
    if _PROG is None:
        _PROG = _build_program()
    return _PROG


def kernel(**inputs) -> tuple:
    global LAST_EXEC_NS
    import ml_dtypes

    from concourse.bass_utils import run_bass_kernel_spmd

    bf = ml_dtypes.bfloat16

    x = np.asarray(inputs["x"], dtype=np.float32)
    src = np.asarray(inputs["src"]).astype(np.int64)
    dst = np.asarray(inputs["dst"]).astype(np.int64)

    # ---- host-side: bake the normalized adjacency (transposed), per hint:
    # edge partitioning by dst == column shards of A_hat^T.
    deg = 1.0 + np.bincount(dst, minlength=N).astype(np.float64)
    dinv = (1.0 / np.sqrt(deg)).astype(np.float32)
    # A'^T = (A + I)^T as exact small-integer edge counts (fp8-representable)
    flat = np.bincount(src * N + dst, minlength=N * N)
    assert flat.max() <= 16, "edge multiplicity too large for exact fp8"
    AT = flat.astype(np.float32).reshape(N, N)
    AT[np.arange(N), np.arange(N)] += 1.0

    w1 = np.concatenate(
        [inputs["enc_a_W1"], inputs["enc_s_W1"], inputs["enc_t_W1"]], axis=1
    )
    w2 = np.concatenate(
        [inputs["enc_a_W2"], inputs["enc_s_W2"], inputs["enc_t_W2"]], axis=1
    )
    def to_sbuf_layout(mat):
        # [N, F] node-major -> [128, KC*F]: row p holds chunks k of node k*128+p
        f = mat.shape[1]
        return np.ascontiguousarray(
            mat.reshape(KC, P, f).transpose(1, 0, 2).reshape(P, KC * f)
        )

    f8 = ml_dtypes.float8_e4m3

    common = {
        "xb": to_sbuf_layout((dinv[:, None] * x).astype(bf)),
        "w1": np.ascontiguousarray(np.asarray(w1, np.float32).astype(bf)),
        "w2": np.ascontiguousarray(np.asarray(w2, np.float32).astype(bf)),
        "aw": np.ascontiguousarray(np.asarray(inputs["att_W"], np.float32).astype(bf)),
        "dwx": np.ascontiguousarray(
            np.asarray(inputs["dec_x_W"], np.float32).astype(bf)
        ),
        "dws": np.ascontiguousarray(
            np.asarray(inputs["dec_s_W"], np.float32).astype(bf)
        ),
    }
    in_maps = []
    for r in range(NCORES):
        m = dict(common)
        m["at"] = to_sbuf_layout(AT[:, r * NB : (r + 1) * NB].astype(f8))
        dblk = dinv[r * NB : (r + 1) * NB]
        m["dv"] = np.ascontiguousarray(dblk.reshape(MT, P).T)
        m["dv2"] = np.ascontiguousarray((dblk * dblk).reshape(MT, P).T)
        m["dvr"] = np.ascontiguousarray(np.broadcast_to(dblk[None, :], (IN, NB)))
        in_maps.append(m)

    nc = _get_program()
    kwargs = {}
    if TRACE:
        kwargs = dict(trace=True, trace_cores=list(range(NCORES)))
    res = run_bass_kernel_spmd(nc, in_maps, core_ids=list(range(NCORES)), **kwargs)
    LAST_EXEC_NS = res.exec_time_ns
    results = res.results

    s_ = np.concatenate([results[r]["s_rows"] for r in range(NCORES)], axis=0)
    x_ = np.concatenate([results[r]["x_rows"] for r in range(NCORES)], axis=0)
    att = np.concatenate(
        [results[r]["att_rows"] for r in range(NCORES)], axis=0
    ).reshape(N, HID, 3)
    return (
        np.asarray(x_, np.float32),
        np.asarray(s_, np.float32),
        np.asarray(att, np.float32),
    )


# revision 13
# speedup vs baseline: 1.5724x; 1.0070x over previous
"""Distributed Trainium2 kernel for the ADAGAD GNN message-passing model.

Model (see problem reference): three 2-layer GCN encoders over a shared
graph, attention-softmax fusion of the three embeddings, two GCN decoder
heads, and a final dense similarity matrix s_ = h_ @ h_.T.

Every GCN conv uses the same symmetric-normalized adjacency with self
loops, A_hat = D^-1/2 (A + I) D^-1/2 (D = 1 + in-degree).  The host
pre-bakes A_hat^T densely in bf16 and column-shards it over the 8 cores
(columns = destination nodes, matching the "partition edges by dst"
sharding).  Each core keeps its 16 MB shard resident in SBUF and runs all
sparse aggregations as dense TensorE matmuls in "outT" form:

    aggT[f, m] = sum_k H[k, f] * A_hatT[k, m]   (lhsT = H chunk, rhs = A_hatT)

which produces feature-major aggregates whose slices feed directly as
lhsT into the small dense-weight matmuls, flipping back to node-major
with no transposes anywhere.  Cross-core exchange is three bf16
AllGathers (H1 after encoder layer 1, h after fusion, h_^T before the
final row-sharded h_ @ h_.T whose 32 MB/core f32 output write is the
memory-roofline term).
"""

import numpy as np

N = 8192
IN = 64
HID = 64
F3 = 3 * HID          # 192
NCORES = 8
NB = N // NCORES      # 1024 rows (dst nodes) per core
P = 128               # partitions
KC = N // P           # 64 contraction chunks
MT = NB // P          # 8 m-tiles per core

TRACE = False         # set by test harness to collect HW exec time
LAST_EXEC_NS = None

_PROG = None


def _build_program():
    import concourse.bass as bass
    import concourse.mybir as mybir
    from concourse import bacc
    from concourse.bass import ds
    from concourse.tile import TileContext

    bf16 = mybir.dt.bfloat16
    f32 = mybir.dt.float32
    f8 = mybir.dt.float8e4
    f8 = mybir.dt.float8e4
    Relu = mybir.ActivationFunctionType.Relu
    Exp = mybir.ActivationFunctionType.Exp
    Copy = mybir.ActivationFunctionType.Copy
    RG = [list(range(NCORES))]

    nc = bacc.Bacc(None, num_devices=NCORES, target_bir_lowering=False, debug=True)

    # host pre-arranged to SBUF layout: [p, k, :] = row k*128+p of the
    # node-major matrix, flattened -> fully contiguous per-partition DMAs
    # A+I edge counts, exact small integers in fp8; D^-1/2 folded into evacs
    at = nc.declare_dram_parameter("at", [P, KC * NB], f8, isOutput=False)
    xb = nc.declare_dram_parameter("xb", [P, KC * IN], bf16, isOutput=False)
    dv = nc.declare_dram_parameter("dv", [P, MT], f32, isOutput=False)
    dv2 = nc.declare_dram_parameter("dv2", [P, MT], f32, isOutput=False)
    dvr = nc.declare_dram_parameter("dvr", [IN, NB], f32, isOutput=False)
    w1 = nc.declare_dram_parameter("w1", [IN, F3], bf16, isOutput=False)
    w2 = nc.declare_dram_parameter("w2", [HID, F3], bf16, isOutput=False)
    aw = nc.declare_dram_parameter("aw", [F3, F3], bf16, isOutput=False)
    dwx = nc.declare_dram_parameter("dwx", [HID, IN], bf16, isOutput=False)
    dws = nc.declare_dram_parameter("dws", [HID, IN], bf16, isOutput=False)

    s_rows = nc.declare_dram_parameter("s_rows", [NB, N], f32, isOutput=True)
    x_rows = nc.declare_dram_parameter("x_rows", [NB, IN], f32, isOutput=True)
    att_rows = nc.declare_dram_parameter("att_rows", [NB, F3], f32, isOutput=True)

    # AG1/AG2 bounces keep the SBUF tile layout: in = [P, MT*F], out adds a
    # leading rank dim; global chunk k = r*MT + m matches at_sb row order.
    ag1_in = nc.dram_tensor("ag1_in", [P, MT * F3], bf16)
    ag1_out = nc.dram_tensor("ag1_out", [NCORES, P, MT * F3], bf16, addr_space="Shared")
    ag2_in = nc.dram_tensor("ag2_in", [P, MT * HID], bf16)
    ag2_out = nc.dram_tensor("ag2_out", [NCORES, P, MT * HID], bf16, addr_space="Shared")
    agw_in = nc.dram_tensor("agw_in", [1, 16], bf16)
    agw_out = nc.dram_tensor("agw_out", [NCORES, 16], bf16, addr_space="Shared")
    ag3_in = nc.dram_tensor("ag3_in", [IN, NB], bf16)
    ag3_out = nc.dram_tensor("ag3_out", [NCORES * IN, NB], bf16, addr_space="Shared")

    def ag(in_t, out_t):
        nc.gpsimd.collective_compute(
            "AllGather",
            mybir.AluOpType.bypass,
            replica_groups=RG,
            ins=[in_t[...]],
            outs=[out_t[...]],
        )

    with TileContext(nc) as tc:
        with (
            tc.tile_pool(name="pat", bufs=1) as pat,
            tc.tile_pool(name="ph", bufs=1) as ph,
            tc.tile_pool(name="pw", bufs=1) as pw,
            tc.tile_pool(name="psm", bufs=1) as psm,
            tc.tile_pool(name="pacc", bufs=1, space="PSUM") as pacc,
            tc.tile_pool(name="prot", bufs=2, space="PSUM") as prot,
        ):
            # warm up the collective path while the big loads stream
            ag(agw_in, agw_out)

            # ---- weights
            w1_sb = pw.tile([IN, F3], bf16, name="w1_sb")
            nc.sync.dma_start(out=w1_sb, in_=w1[:, :])
            w2_sb = pw.tile([HID, F3], bf16, name="w2_sb")
            nc.sync.dma_start(out=w2_sb, in_=w2[:, :])
            aw_hi = pw.tile([P, F3], bf16, name="aw_hi")
            nc.sync.dma_start(out=aw_hi, in_=aw[0:P, :])
            aw_lo = pw.tile([F3 - P, F3], bf16, name="aw_lo")
            nc.sync.dma_start(out=aw_lo, in_=aw[P:F3, :])
            dwx_sb = pw.tile([HID, IN], bf16, name="dwx_sb")
            nc.sync.dma_start(out=dwx_sb, in_=dwx[:, :])
            dws_sb = pw.tile([HID, IN], bf16, name="dws_sb")
            nc.sync.dma_start(out=dws_sb, in_=dws[:, :])

            # ---- stage A: adjacency + x loads, SpMM1: a0T = (A_hat x)^T
            x_sb = ph.tile([P, KC, IN], bf16, name="x_sb", tag="hbuf")
            nc.scalar.dma_start(out=x_sb, in_=xb[:, :])
            dv_sb = pw.tile([P, MT], f32, name="dv_sb")
            nc.scalar.dma_start(out=dv_sb, in_=dv[:, :])
            dv2_sb = pw.tile([P, MT], f32, name="dv2_sb")
            nc.scalar.dma_start(out=dv2_sb, in_=dv2[:, :])
            dvr_sb = pw.tile([IN, NB], f32, name="dvr_sb")
            nc.scalar.dma_start(out=dvr_sb, in_=dvr[:, :])
            at_sb = pat.tile([P, KC, NB], f8, name="at_sb")
            for g in range(8):
                eng = nc.sync if g % 2 == 0 else nc.scalar
                eng.dma_start(
                    out=at_sb[:, g * 8 : (g + 1) * 8, :],
                    in_=at[:, g * 8 * NB : (g + 1) * 8 * NB],
                )

            a0_ps0 = pacc.tile([IN, 512], f32, name="a0_ps0", tag="accA")
            a0_ps1 = pacc.tile([IN, 512], f32, name="a0_ps1", tag="accB")
            for k in range(KC):
                st, sp = (k == 0), (k == KC - 1)
                nc.tensor.matmul(
                    a0_ps0, x_sb[:, k, :], at_sb[:, k, 0:512], start=st, stop=sp
                )
                nc.tensor.matmul(
                    a0_ps1, x_sb[:, k, :], at_sb[:, k, 512:1024], start=st, stop=sp
                )
            a0_sb = psm.tile([IN, NB], bf16, name="a0_sb")
            nc.vector.tensor_copy(out=a0_sb[:, 0:512], in_=a0_ps0)
            nc.vector.tensor_copy(out=a0_sb[:, 512:1024], in_=a0_ps1)

            # ---- stage B: u1 = dinv*relu(dinv*(a0 @ W1)) = relu(dinv^2 * .)
            h1_sb = psm.tile([P, MT, F3], bf16, name="h1_sb")
            for m in range(MT):
                ps = prot.tile([P, F3], f32, name="h1_ps", tag="rot")
                nc.tensor.matmul(
                    ps, a0_sb[:, ds(m * P, P)], w1_sb, start=True, stop=True
                )
                nc.scalar.activation(
                    h1_sb[:, m, :], ps, Relu, scale=dv2_sb[:, m : m + 1]
                )
            nc.sync.dma_start(out=ag1_in[:, :], in_=h1_sb)

            # ---- AG1
            ag(ag1_in, ag1_out)
            H1_sb = ph.tile([P, KC, F3], bf16, name="H1_sb", tag="hbuf")
            ag1v = ag1_out.rearrange("r p q -> p r q")
            nc.sync.dma_start(out=H1_sb[:, 0:8, :], in_=ag1v[:, 0:1, :])
            nc.scalar.dma_start(out=H1_sb[:, 8:KC, :], in_=ag1v[:, 1:NCORES, :])

            # ---- stage D: SpMM2: a1T = (A_hat H1)^T, two stationary pieces
            a1h0 = pacc.tile([P, 512], f32, name="a1h0", tag="accA")
            a1h1 = pacc.tile([P, 512], f32, name="a1h1", tag="accB")
            a1l0 = pacc.tile([F3 - P, 512], f32, name="a1l0", tag="accC")
            a1l1 = pacc.tile([F3 - P, 512], f32, name="a1l1", tag="accD")
            for k in range(KC):
                st, sp = (k == 0), (k == KC - 1)
                hi = H1_sb[:, k, 0:P]
                nc.tensor.matmul(a1h0, hi, at_sb[:, k, 0:512], start=st, stop=sp)
                nc.tensor.matmul(a1h1, hi, at_sb[:, k, 512:1024], start=st, stop=sp)
            for k in range(KC):
                st, sp = (k == 0), (k == KC - 1)
                lo = H1_sb[:, k, P:F3]
                nc.tensor.matmul(a1l0, lo, at_sb[:, k, 0:512], start=st, stop=sp)
                nc.tensor.matmul(a1l1, lo, at_sb[:, k, 512:1024], start=st, stop=sp)
            # evacuate into per-encoder base-0 tiles (partition-shifted copies)
            a1_sb = [
                psm.tile([IN, NB], bf16, name=f"a1_sb{e}", tag=f"a1_sb{e}")
                for e in range(3)
            ]
            nc.vector.tensor_copy(out=a1_sb[0][:, 0:512], in_=a1h0[0:64, :])
            nc.vector.tensor_copy(out=a1_sb[0][:, 512:1024], in_=a1h1[0:64, :])
            nc.vector.tensor_copy(out=a1_sb[1][:, 0:512], in_=a1h0[64:128, :])
            nc.vector.tensor_copy(out=a1_sb[1][:, 512:1024], in_=a1h1[64:128, :])
            nc.vector.tensor_copy(out=a1_sb[2][:, 0:512], in_=a1l0)
            nc.vector.tensor_copy(out=a1_sb[2][:, 512:1024], in_=a1l1)

            def a1_enc(e):
                # feature-major agg1 slice for encoder e: [64, NB], base 0
                return a1_sb[e][:, :]

            # ---- stage E: cT (feature-major relu'd concat) + he (node-major)
            cT_hi = psm.tile([P, NB], bf16, name="cT_hi")
            cT_lo = psm.tile([F3 - P, NB], bf16, name="cT_lo")
            for e in range(3):
                for i in range(2):
                    ps = prot.tile([IN, 512], f32, name="ct_ps", tag="rot")
                    nc.tensor.matmul(
                        ps,
                        w2_sb[:, ds(e * HID, HID)],
                        a1_enc(e)[:, ds(i * 512, 512)],
                        start=True,
                        stop=True,
                    )
                    if e == 0:
                        dst = cT_hi[0:64, ds(i * 512, 512)]
                    elif e == 1:
                        dst = cT_hi[64:128, ds(i * 512, 512)]
                    else:
                        dst = cT_lo[0:64, ds(i * 512, 512)]
                    # partition-shifted relu evac (base 0 -> base 64 for e=1)
                    nc.vector.tensor_relu(out=dst, in_=ps)

            # att_in = c @ att_W, node-major, evacuated to att_sb
            att_sb = psm.tile([P, MT, F3], f32, name="att_sb")
            for m in range(MT):
                ps = prot.tile([P, F3], f32, name="att_ps", tag="rot")
                nc.tensor.matmul(
                    ps, cT_hi[:, ds(m * P, P)], aw_hi, start=True, stop=False
                )
                nc.tensor.matmul(
                    ps, cT_lo[:, ds(m * P, P)], aw_lo, start=False, stop=True
                )
                nc.scalar.activation(
                    att_sb[:, m, :], ps, Copy, scale=dv_sb[:, m : m + 1]
                )

            he_sb = psm.tile([P, 3, MT, IN], f32, name="he_sb")
            for e in range(3):
                for m in range(MT):
                    ps = prot.tile([P, IN], f32, name="he_ps", tag="rot")
                    nc.tensor.matmul(
                        ps,
                        a1_enc(e)[:, ds(m * P, P)],
                        w2_sb[:, ds(e * HID, HID)],
                        start=True,
                        stop=True,
                    )
                    nc.scalar.activation(
                        he_sb[:, e, m, :], ps, Relu, scale=dv_sb[:, m : m + 1]
                    )

            # ---- stage F: softmax over j (groups of 3) in place, then fuse
            attv = att_sb.rearrange("p m (h j) -> p m j h", j=3)
            mx = psm.tile([P, MT, IN], f32, name="mx", tag="ftmp", bufs=2)
            nc.vector.tensor_max(out=mx, in0=attv[:, :, 0, :], in1=attv[:, :, 1, :])
            nc.vector.tensor_max(out=mx, in0=mx, in1=attv[:, :, 2, :])
            for j in range(3):
                nc.vector.tensor_sub(
                    out=attv[:, :, j, :], in0=attv[:, :, j, :], in1=mx
                )
            for j in range(3):
                nc.scalar.activation(attv[:, :, j, :], attv[:, :, j, :], Exp)
            ssum = psm.tile([P, MT, IN], f32, name="ssum", tag="ftmp", bufs=2)
            nc.vector.tensor_add(
                out=ssum, in0=attv[:, :, 0, :], in1=attv[:, :, 1, :]
            )
            nc.vector.tensor_add(out=ssum, in0=ssum, in1=attv[:, :, 2, :])
            rcp = psm.tile([P, MT, IN], f32, name="rcp", tag="ftmp", bufs=2)
            nc.vector.reciprocal_approx_fast(out=rcp, in_=ssum)
            for j in range(3):
                nc.vector.tensor_mul(
                    out=attv[:, :, j, :], in0=attv[:, :, j, :], in1=rcp
                )
            nc.sync.dma_start(
                out=att_rows.rearrange("(m p) f -> p m f", p=P), in_=att_sb
            )

            hacc = psm.tile([P, MT, IN], f32, name="hacc", tag="ftmp", bufs=2)
            htmp = psm.tile([P, MT, IN], f32, name="htmp", tag="ftmp", bufs=2)
            nc.vector.tensor_mul(out=hacc, in0=he_sb[:, 0], in1=attv[:, :, 0, :])
            nc.vector.tensor_mul(out=htmp, in0=he_sb[:, 1], in1=attv[:, :, 1, :])
            nc.vector.tensor_add(out=hacc, in0=hacc, in1=htmp)
            nc.vector.tensor_mul(out=htmp, in0=he_sb[:, 2], in1=attv[:, :, 2, :])
            nc.vector.tensor_add(out=hacc, in0=hacc, in1=htmp)
            h_sb = psm.tile([P, MT, IN], bf16, name="h_sb")
            for m in range(MT):
                nc.vector.tensor_scalar_mul(
                    h_sb[:, m, :], hacc[:, m, :], dv_sb[:, m : m + 1]
                )
            nc.sync.dma_start(out=ag2_in[:, :], in_=h_sb)

            # ---- AG2 + reload
            ag(ag2_in, ag2_out)
            H2_sb = ph.tile([P, KC, IN], bf16, name="H2_sb", tag="hbuf")
            ag2v = ag2_out.rearrange("r p q -> p r q")
            nc.sync.dma_start(out=H2_sb[:, 0:8, :], in_=ag2v[:, 0:1, :])
            nc.scalar.dma_start(out=H2_sb[:, 8:KC, :], in_=ag2v[:, 1:NCORES, :])

            # ---- stage H: SpMM3: a2T = (A_hat h)^T
            a2_ps0 = pacc.tile([IN, 512], f32, name="a2_ps0", tag="accA")
            a2_ps1 = pacc.tile([IN, 512], f32, name="a2_ps1", tag="accB")
            for k in range(KC):
                st, sp = (k == 0), (k == KC - 1)
                nc.tensor.matmul(
                    a2_ps0, H2_sb[:, k, :], at_sb[:, k, 0:512], start=st, stop=sp
                )
                nc.tensor.matmul(
                    a2_ps1, H2_sb[:, k, :], at_sb[:, k, 512:1024], start=st, stop=sp
                )
            a2_sb = psm.tile([IN, NB], bf16, name="a2_sb")
            nc.vector.tensor_copy(out=a2_sb[:, 0:512], in_=a2_ps0)
            nc.vector.tensor_copy(out=a2_sb[:, 512:1024], in_=a2_ps1)

            # ---- stage I: decoder heads
            xo_sb = psm.tile([P, MT, IN], f32, name="xo_sb")
            for m in range(MT):
                ps = prot.tile([P, IN], f32, name="xo_ps", tag="rot")
                nc.tensor.matmul(
                    ps, a2_sb[:, ds(m * P, P)], dwx_sb, start=True, stop=True
                )
                nc.scalar.activation(
                    xo_sb[:, m, :], ps, Copy, scale=dv_sb[:, m : m + 1]
                )
            nc.sync.dma_start(
                out=x_rows.rearrange("(m p) f -> p m f", p=P), in_=xo_sb
            )

            hT_ps0 = pacc.tile([IN, 512], f32, name="hT_ps0", tag="accA")
            hT_ps1 = pacc.tile([IN, 512], f32, name="hT_ps1", tag="accB")
            nc.tensor.matmul(hT_ps0, dws_sb, a2_sb[:, 0:512], start=True, stop=True)
            nc.tensor.matmul(
                hT_ps1, dws_sb, a2_sb[:, 512:1024], start=True, stop=True
            )
            hT_sb = psm.tile([IN, NB], bf16, name="hT_sb")
            nc.vector.tensor_mul(
                out=hT_sb[:, 0:512], in0=hT_ps0, in1=dvr_sb[:, 0:512]
            )
            nc.vector.tensor_mul(
                out=hT_sb[:, 512:1024], in0=hT_ps1, in1=dvr_sb[:, 512:1024]
            )
            nc.sync.dma_start(out=ag3_in[:, :], in_=hT_sb)

            # ---- AG3 + reload h_^T for all nodes
            ag(ag3_in, ag3_out)
            hTf_sb = ph.tile([IN, NCORES, NB], bf16, name="hTf_sb", tag="hbuf")
            nc.sync.dma_start(
                out=hTf_sb, in_=ag3_out.rearrange("(r f) m -> f r m", f=IN)
            )

            # ---- stage K: s_ rows = h_rows @ h_full^T (32 MB f32 out)
            dma_engines = [nc.sync, nc.scalar, nc.gpsimd]
            for m in range(MT):
                lhsT = hT_sb[:, ds(m * P, P)]
                for g in range(8):  # groups of 2 n-tiles -> one 512 KB DMA
                    ev = psm.tile([P, 1024], f32, name="s_ev", tag="sev", bufs=6)
                    for i in range(2):
                        n = g * 2 + i
                        ps = pacc.tile(
                            [P, 512], f32, name="s_ps",
                            tag=["accA", "accB", "accC", "accD"][n % 4],
                        )
                        nc.tensor.matmul(
                            ps,
                            lhsT,
                            hTf_sb[:, n // 2, ds((n % 2) * 512, 512)],
                            start=True,
                            stop=True,
                        )
                        if i % 2 == 0:
                            nc.scalar.copy(out=ev[:, ds(i * 512, 512)], in_=ps)
                        else:
                            nc.vector.tensor_copy(
                                out=ev[:, ds(i * 512, 512)], in_=ps
                            )
                    dma_engines[g % 3].dma_start(
                        out=s_rows[ds(m * P, P), ds(g * 1024, 1024)], in_=ev
                    )

    nc.finalize()
    return nc


def _get_program():
    global _PROG
    if _PROG is None:
        _PROG = _build_program()
    return _PROG


def kernel(**inputs) -> tuple:
    global LAST_EXEC_NS
    import ml_dtypes

    from concourse.bass_utils import run_bass_kernel_spmd

    bf = ml_dtypes.bfloat16

    x = np.asarray(inputs["x"], dtype=np.float32)
    src = np.asarray(inputs["src"]).astype(np.int64)
    dst = np.asarray(inputs["dst"]).astype(np.int64)

    # ---- host-side: bake the normalized adjacency (transposed), per hint:
    # edge partitioning by dst == column shards of A_hat^T.
    deg = 1.0 + np.bincount(dst, minlength=N).astype(np.float64)
    dinv = (1.0 / np.sqrt(deg)).astype(np.float32)
    # A'^T = (A + I)^T as exact small-integer edge counts (fp8-representable)
    flat = np.bincount(src * N + dst, minlength=N * N)
    assert flat.max() <= 16, "edge multiplicity too large for exact fp8"
    AT = flat.astype(np.float32).reshape(N, N)
    AT[np.arange(N), np.arange(N)] += 1.0

    w1 = np.concatenate(
        [inputs["enc_a_W1"], inputs["enc_s_W1"], inputs["enc_t_W1"]], axis=1
    )
    w2 = np.concatenate(
        [inputs["enc_a_W2"], inputs["enc_s_W2"], inputs["enc_t_W2"]], axis=1
    )
    def to_sbuf_layout(mat):
        # [N, F] node-major -> [128, KC*F]: row p holds chunks k of node k*128+p
        f = mat.shape[1]
        return np.ascontiguousarray(
            mat.reshape(KC, P, f).transpose(1, 0, 2).reshape(P, KC * f)
        )

    f8 = ml_dtypes.float8_e4m3

    common = {
        "xb": to_sbuf_layout((dinv[:, None] * x).astype(bf)),
        "w1": np.ascontiguousarray(np.asarray(w1, np.float32).astype(bf)),
        "w2": np.ascontiguousarray(np.asarray(w2, np.float32).astype(bf)),
        "aw": np.ascontiguousarray(np.asarray(inputs["att_W"], np.float32).astype(bf)),
        "dwx": np.ascontiguousarray(
            np.asarray(inputs["dec_x_W"], np.float32).astype(bf)
        ),
        "dws": np.ascontiguousarray(
            np.asarray(inputs["dec_s_W"], np.float32).astype(bf)
        ),
    }
    in_maps = []
    for r in range(NCORES):
        m = dict(common)
        m["at"] = to_sbuf_layout(AT[:, r * NB : (r + 1) * NB].astype(f8))
        dblk = dinv[r * NB : (r + 1) * NB]
        m["dv"] = np.ascontiguousarray(dblk.reshape(MT, P).T)
        m["dv2"] = np.ascontiguousarray((dblk * dblk).reshape(MT, P).T)
        m["dvr"] = np.ascontiguousarray(np.broadcast_to(dblk[None, :], (IN, NB)))
        in_maps.append(m)

    nc = _get_program()
    kwargs = {}
    if TRACE:
        kwargs = dict(trace=True, trace_cores=list(range(NCORES)))
    res = run_bass_kernel_spmd(nc, in_maps, core_ids=list(range(NCORES)), **kwargs)
    LAST_EXEC_NS = res.exec_time_ns
    results = res.results

    s_ = np.concatenate([results[r]["s_rows"] for r in range(NCORES)], axis=0)
    x_ = np.concatenate([results[r]["x_rows"] for r in range(NCORES)], axis=0)
    att = np.concatenate(
        [results[r]["att_rows"] for r in range(NCORES)], axis=0
    ).reshape(N, HID, 3)
    return (
        np.asarray(x_, np.float32),
        np.asarray(s_, np.float32),
        np.asarray(att, np.float32),
    )


# revision 15
# speedup vs baseline: 1.6260x; 1.0341x over previous
"""Distributed Trainium2 kernel for the ADAGAD GNN message-passing model.

Model (see problem reference): three 2-layer GCN encoders over a shared
graph, attention-softmax fusion of the three embeddings, two GCN decoder
heads, and a final dense similarity matrix s_ = h_ @ h_.T.

Every GCN conv uses the same symmetric-normalized adjacency with self
loops, A_hat = D^-1/2 (A + I) D^-1/2 (D = 1 + in-degree).  The host
pre-bakes A_hat^T densely in bf16 and column-shards it over the 8 cores
(columns = destination nodes, matching the "partition edges by dst"
sharding).  Each core keeps its 16 MB shard resident in SBUF and runs all
sparse aggregations as dense TensorE matmuls in "outT" form:

    aggT[f, m] = sum_k H[k, f] * A_hatT[k, m]   (lhsT = H chunk, rhs = A_hatT)

which produces feature-major aggregates whose slices feed directly as
lhsT into the small dense-weight matmuls, flipping back to node-major
with no transposes anywhere.  Cross-core exchange is three bf16
AllGathers (H1 after encoder layer 1, h after fusion, h_^T before the
final row-sharded h_ @ h_.T whose 32 MB/core f32 output write is the
memory-roofline term).
"""

import numpy as np

N = 8192
IN = 64
HID = 64
F3 = 3 * HID          # 192
NCORES = 8
NB = N // NCORES      # 1024 rows (dst nodes) per core
P = 128               # partitions
KC = N // P           # 64 contraction chunks
MT = NB // P          # 8 m-tiles per core

TRACE = False         # set by test harness to collect HW exec time
LAST_EXEC_NS = None

_PROG = None


def _build_program():
    import concourse.bass as bass
    import concourse.mybir as mybir
    from concourse import bacc
    from concourse.bass import ds
    from concourse.tile import TileContext

    bf16 = mybir.dt.bfloat16
    f32 = mybir.dt.float32
    f8 = mybir.dt.float8e4
    f8 = mybir.dt.float8e4
    Relu = mybir.ActivationFunctionType.Relu
    Exp = mybir.ActivationFunctionType.Exp
    Copy = mybir.ActivationFunctionType.Copy
    RG = [list(range(NCORES))]

    nc = bacc.Bacc(None, num_devices=NCORES, target_bir_lowering=False, debug=True)

    # host pre-arranged to SBUF layout: [p, k, :] = row k*128+p of the
    # node-major matrix, flattened -> fully contiguous per-partition DMAs
    # A+I edge counts, exact small integers in fp8; D^-1/2 folded into evacs
    at = nc.declare_dram_parameter("at", [P, KC * NB], f8, isOutput=False)
    xb = nc.declare_dram_parameter("xb", [P, KC * IN], bf16, isOutput=False)
    dv = nc.declare_dram_parameter("dv", [P, MT], f32, isOutput=False)
    dv2 = nc.declare_dram_parameter("dv2", [P, MT], f32, isOutput=False)
    dvr = nc.declare_dram_parameter("dvr", [IN, NB], f32, isOutput=False)
    w1 = nc.declare_dram_parameter("w1", [IN, F3], bf16, isOutput=False)
    w2 = nc.declare_dram_parameter("w2", [HID, F3], bf16, isOutput=False)
    aw = nc.declare_dram_parameter("aw", [F3, F3], bf16, isOutput=False)
    dwx = nc.declare_dram_parameter("dwx", [HID, IN], bf16, isOutput=False)
    dws = nc.declare_dram_parameter("dws", [HID, IN], bf16, isOutput=False)

    s_rows = nc.declare_dram_parameter("s_rows", [NB, N], f32, isOutput=True)
    x_rows = nc.declare_dram_parameter("x_rows", [NB, IN], f32, isOutput=True)
    att_rows = nc.declare_dram_parameter("att_rows", [NB, F3], f32, isOutput=True)

    # AG1/AG2 bounces keep the SBUF tile layout: in = [P, MT*F], out adds a
    # leading rank dim; global chunk k = r*MT + m matches at_sb row order.
    ag1_in = nc.dram_tensor("ag1_in", [P, MT * F3], bf16)
    ag1_out = nc.dram_tensor("ag1_out", [NCORES, P, MT * F3], bf16, addr_space="Shared")
    ag2_in = nc.dram_tensor("ag2_in", [P, MT * HID], bf16)
    ag2_out = nc.dram_tensor("ag2_out", [NCORES, P, MT * HID], bf16, addr_space="Shared")
    agw_in = nc.dram_tensor("agw_in", [1, 16], bf16)
    agw_out = nc.dram_tensor("agw_out", [NCORES, 16], bf16, addr_space="Shared")
    ag3_in = nc.dram_tensor("ag3_in", [IN, NB], bf16)
    ag3_out = nc.dram_tensor("ag3_out", [NCORES * IN, NB], bf16, addr_space="Shared")

    def ag(in_t, out_t):
        nc.gpsimd.collective_compute(
            "AllGather",
            mybir.AluOpType.bypass,
            replica_groups=RG,
            ins=[in_t[...]],
            outs=[out_t[...]],
        )

    with TileContext(nc) as tc:
        with (
            tc.tile_pool(name="pat", bufs=1) as pat,
            tc.tile_pool(name="ph", bufs=1) as ph,
            tc.tile_pool(name="pw", bufs=1) as pw,
            tc.tile_pool(name="psm", bufs=1) as psm,
            tc.tile_pool(name="pacc", bufs=1, space="PSUM") as pacc,
            tc.tile_pool(name="prot", bufs=2, space="PSUM") as prot,
        ):
            # warm up the collective path while the big loads stream
            ag(agw_in, agw_out)

            # warm the PE clock (HAM) before real matmuls arrive
            junk_sb = pw.tile([P, 512], bf16, name="junk_sb")
            nc.vector.memset(junk_sb, 0.0)
            junk_ps = prot.tile([P, 512], f32, name="junk_ps", tag="rot")
            for _ in range(10):
                nc.tensor.matmul(
                    junk_ps, junk_sb[:, 0:P], junk_sb, start=True, stop=True
                )

            # ---- weights
            w1_sb = pw.tile([IN, F3], bf16, name="w1_sb")
            nc.sync.dma_start(out=w1_sb, in_=w1[:, :])
            w2_sb = pw.tile([HID, F3], bf16, name="w2_sb")
            nc.sync.dma_start(out=w2_sb, in_=w2[:, :])
            aw_hi = pw.tile([P, F3], bf16, name="aw_hi")
            nc.sync.dma_start(out=aw_hi, in_=aw[0:P, :])
            aw_lo = pw.tile([F3 - P, F3], bf16, name="aw_lo")
            nc.sync.dma_start(out=aw_lo, in_=aw[P:F3, :])
            dwx_sb = pw.tile([HID, IN], bf16, name="dwx_sb")
            nc.sync.dma_start(out=dwx_sb, in_=dwx[:, :])
            dws_sb = pw.tile([HID, IN], bf16, name="dws_sb")
            nc.sync.dma_start(out=dws_sb, in_=dws[:, :])

            # ---- stage A: adjacency + x loads, SpMM1: a0T = (A_hat x)^T
            x_sb = ph.tile([P, KC, IN], bf16, name="x_sb", tag="hbuf")
            nc.scalar.dma_start(out=x_sb, in_=xb[:, :])

            at_sb = pat.tile([P, KC, NB], f8, name="at_sb")
            for g in range(8):
                eng = nc.sync if g % 2 == 0 else nc.scalar
                eng.dma_start(
                    out=at_sb[:, g * 8 : (g + 1) * 8, :],
                    in_=at[:, g * 8 * NB : (g + 1) * 8 * NB],
                )
            dv_sb = pw.tile([P, MT], f32, name="dv_sb")
            nc.scalar.dma_start(out=dv_sb, in_=dv[:, :])
            dv2_sb = pw.tile([P, MT], f32, name="dv2_sb")
            nc.scalar.dma_start(out=dv2_sb, in_=dv2[:, :])
            dvr_sb = pw.tile([IN, NB], f32, name="dvr_sb")
            nc.scalar.dma_start(out=dvr_sb, in_=dvr[:, :])

            a0_ps0 = pacc.tile([IN, 512], f32, name="a0_ps0", tag="accA")
            a0_ps1 = pacc.tile([IN, 512], f32, name="a0_ps1", tag="accB")
            for k in range(KC):
                st, sp = (k == 0), (k == KC - 1)
                nc.tensor.matmul(
                    a0_ps0, x_sb[:, k, :], at_sb[:, k, 0:512], start=st, stop=sp
                )
                nc.tensor.matmul(
                    a0_ps1, x_sb[:, k, :], at_sb[:, k, 512:1024], start=st, stop=sp
                )
            a0_sb = psm.tile([IN, NB], bf16, name="a0_sb")
            nc.vector.tensor_copy(out=a0_sb[:, 0:512], in_=a0_ps0)
            nc.vector.tensor_copy(out=a0_sb[:, 512:1024], in_=a0_ps1)

            # ---- stage B: u1 = dinv*relu(dinv*(a0 @ W1)) = relu(dinv^2 * .)
            h1_sb = psm.tile([P, MT, F3], bf16, name="h1_sb")
            for m in range(MT):
                ps = prot.tile([P, F3], f32, name="h1_ps", tag="rot")
                nc.tensor.matmul(
                    ps, a0_sb[:, ds(m * P, P)], w1_sb, start=True, stop=True
                )
                nc.scalar.activation(
                    h1_sb[:, m, :], ps, Relu, scale=dv2_sb[:, m : m + 1]
                )
            nc.sync.dma_start(out=ag1_in[:, :], in_=h1_sb)

            # ---- AG1
            ag(ag1_in, ag1_out)
            H1_sb = ph.tile([P, KC, F3], bf16, name="H1_sb", tag="hbuf")
            ag1v = ag1_out.rearrange("r p q -> p r q")
            nc.sync.dma_start(out=H1_sb[:, 0:8, :], in_=ag1v[:, 0:1, :])
            nc.scalar.dma_start(out=H1_sb[:, 8:KC, :], in_=ag1v[:, 1:NCORES, :])

            # ---- stage D: SpMM2: a1T = (A_hat H1)^T, two stationary pieces
            a1h0 = pacc.tile([P, 512], f32, name="a1h0", tag="accA")
            a1h1 = pacc.tile([P, 512], f32, name="a1h1", tag="accB")
            a1l0 = pacc.tile([F3 - P, 512], f32, name="a1l0", tag="accC")
            a1l1 = pacc.tile([F3 - P, 512], f32, name="a1l1", tag="accD")
            for k in range(KC):
                st, sp = (k == 0), (k == KC - 1)
                hi = H1_sb[:, k, 0:P]
                nc.tensor.matmul(a1h0, hi, at_sb[:, k, 0:512], start=st, stop=sp)
                nc.tensor.matmul(a1h1, hi, at_sb[:, k, 512:1024], start=st, stop=sp)
            for k in range(KC):
                st, sp = (k == 0), (k == KC - 1)
                lo = H1_sb[:, k, P:F3]
                nc.tensor.matmul(a1l0, lo, at_sb[:, k, 0:512], start=st, stop=sp)
                nc.tensor.matmul(a1l1, lo, at_sb[:, k, 512:1024], start=st, stop=sp)
            # evacuate into per-encoder base-0 tiles (partition-shifted copies)
            a1_sb = [
                psm.tile([IN, NB], bf16, name=f"a1_sb{e}", tag=f"a1_sb{e}")
                for e in range(3)
            ]
            nc.vector.tensor_copy(out=a1_sb[0][:, 0:512], in_=a1h0[0:64, :])
            nc.vector.tensor_copy(out=a1_sb[0][:, 512:1024], in_=a1h1[0:64, :])
            nc.vector.tensor_copy(out=a1_sb[1][:, 0:512], in_=a1h0[64:128, :])
            nc.vector.tensor_copy(out=a1_sb[1][:, 512:1024], in_=a1h1[64:128, :])
            nc.vector.tensor_copy(out=a1_sb[2][:, 0:512], in_=a1l0)
            nc.vector.tensor_copy(out=a1_sb[2][:, 512:1024], in_=a1l1)

            def a1_enc(e):
                # feature-major agg1 slice for encoder e: [64, NB], base 0
                return a1_sb[e][:, :]

            # ---- stage E: cT (feature-major relu'd concat) + he (node-major)
            cT_hi = psm.tile([P, NB], bf16, name="cT_hi")
            cT_lo = psm.tile([F3 - P, NB], bf16, name="cT_lo")
            for e in range(3):
                for i in range(2):
                    ps = prot.tile([IN, 512], f32, name="ct_ps", tag="rot")
                    nc.tensor.matmul(
                        ps,
                        w2_sb[:, ds(e * HID, HID)],
                        a1_enc(e)[:, ds(i * 512, 512)],
                        start=True,
                        stop=True,
                    )
                    if e == 0:
                        dst = cT_hi[0:64, ds(i * 512, 512)]
                    elif e == 1:
                        dst = cT_hi[64:128, ds(i * 512, 512)]
                    else:
                        dst = cT_lo[0:64, ds(i * 512, 512)]
                    # partition-shifted relu evac (base 0 -> base 64 for e=1)
                    nc.vector.tensor_relu(out=dst, in_=ps)

            # att_in = c @ att_W, node-major, evacuated to att_sb
            att_sb = psm.tile([P, MT, F3], f32, name="att_sb")
            for m in range(MT):
                ps = prot.tile([P, F3], f32, name="att_ps", tag="rot")
                nc.tensor.matmul(
                    ps, cT_hi[:, ds(m * P, P)], aw_hi, start=True, stop=False
                )
                nc.tensor.matmul(
                    ps, cT_lo[:, ds(m * P, P)], aw_lo, start=False, stop=True
                )
                nc.scalar.activation(
                    att_sb[:, m, :], ps, Copy, scale=dv_sb[:, m : m + 1]
                )

            he_sb = psm.tile([P, 3, MT, IN], f32, name="he_sb")
            for e in range(3):
                for m in range(MT):
                    ps = prot.tile([P, IN], f32, name="he_ps", tag="rot")
                    nc.tensor.matmul(
                        ps,
                        a1_enc(e)[:, ds(m * P, P)],
                        w2_sb[:, ds(e * HID, HID)],
                        start=True,
                        stop=True,
                    )
                    nc.scalar.activation(
                        he_sb[:, e, m, :], ps, Relu, scale=dv_sb[:, m : m + 1]
                    )

            # ---- stage F: softmax over j (groups of 3) in place, then fuse
            attv = att_sb.rearrange("p m (h j) -> p m j h", j=3)
            for j in range(3):
                nc.scalar.activation(attv[:, :, j, :], attv[:, :, j, :], Exp)
            ssum = psm.tile([P, MT, IN], f32, name="ssum", tag="ftmp", bufs=3)
            nc.vector.tensor_add(
                out=ssum, in0=attv[:, :, 0, :], in1=attv[:, :, 1, :]
            )
            nc.vector.tensor_add(out=ssum, in0=ssum, in1=attv[:, :, 2, :])
            rcp = psm.tile([P, MT, IN], f32, name="rcp", tag="ftmp", bufs=3)
            nc.vector.reciprocal_approx_fast(out=rcp, in_=ssum)

            # h-combine on the UNnormalized exps; normalize once at the end:
            # u2 = dinv * (sum_e he*exp_e) * rcp
            hacc = psm.tile([P, MT, IN], f32, name="hacc", tag="ftmp", bufs=3)
            htmp = psm.tile([P, MT, IN], f32, name="htmp", tag="ftmp", bufs=3)
            nc.vector.tensor_mul(out=hacc, in0=he_sb[:, 0], in1=attv[:, :, 0, :])
            nc.vector.tensor_mul(out=htmp, in0=he_sb[:, 1], in1=attv[:, :, 1, :])
            nc.vector.tensor_add(out=hacc, in0=hacc, in1=htmp)
            nc.vector.tensor_mul(out=htmp, in0=he_sb[:, 2], in1=attv[:, :, 2, :])
            nc.vector.tensor_add(out=hacc, in0=hacc, in1=htmp)
            nc.vector.tensor_mul(out=hacc, in0=hacc, in1=rcp)
            h_sb = psm.tile([P, MT, IN], bf16, name="h_sb")
            for m in range(MT):
                nc.vector.tensor_scalar_mul(
                    h_sb[:, m, :], hacc[:, m, :], dv_sb[:, m : m + 1]
                )
            nc.sync.dma_start(out=ag2_in[:, :], in_=h_sb)

            # att output normalization — off the AG2 critical path
            for j in range(3):
                nc.vector.tensor_mul(
                    out=attv[:, :, j, :], in0=attv[:, :, j, :], in1=rcp
                )
            nc.sync.dma_start(
                out=att_rows.rearrange("(m p) f -> p m f", p=P), in_=att_sb
            )
            # ---- AG2 + reload
            ag(ag2_in, ag2_out)
            H2_sb = ph.tile([P, KC, IN], bf16, name="H2_sb", tag="hbuf")
            ag2v = ag2_out.rearrange("r p q -> p r q")
            nc.sync.dma_start(out=H2_sb[:, 0:8, :], in_=ag2v[:, 0:1, :])
            nc.scalar.dma_start(out=H2_sb[:, 8:KC, :], in_=ag2v[:, 1:NCORES, :])

            # ---- stage H: SpMM3: a2T = (A_hat h)^T
            a2_ps0 = pacc.tile([IN, 512], f32, name="a2_ps0", tag="accA")
            a2_ps1 = pacc.tile([IN, 512], f32, name="a2_ps1", tag="accB")
            for k in range(KC):
                st, sp = (k == 0), (k == KC - 1)
                nc.tensor.matmul(
                    a2_ps0, H2_sb[:, k, :], at_sb[:, k, 0:512], start=st, stop=sp
                )
                nc.tensor.matmul(
                    a2_ps1, H2_sb[:, k, :], at_sb[:, k, 512:1024], start=st, stop=sp
                )
            a2_sb = psm.tile([IN, NB], bf16, name="a2_sb")
            nc.vector.tensor_copy(out=a2_sb[:, 0:512], in_=a2_ps0)
            nc.vector.tensor_copy(out=a2_sb[:, 512:1024], in_=a2_ps1)

            # ---- stage I: h_^T first so AG3 launches ASAP
            hT_ps0 = pacc.tile([IN, 512], f32, name="hT_ps0", tag="accA")
            hT_ps1 = pacc.tile([IN, 512], f32, name="hT_ps1", tag="accB")
            nc.tensor.matmul(hT_ps0, dws_sb, a2_sb[:, 0:512], start=True, stop=True)
            nc.tensor.matmul(
                hT_ps1, dws_sb, a2_sb[:, 512:1024], start=True, stop=True
            )
            hT_sb = psm.tile([IN, NB], bf16, name="hT_sb")
            nc.vector.tensor_mul(
                out=hT_sb[:, 0:512], in0=hT_ps0, in1=dvr_sb[:, 0:512]
            )
            nc.vector.tensor_mul(
                out=hT_sb[:, 512:1024], in0=hT_ps1, in1=dvr_sb[:, 512:1024]
            )
            nc.sync.dma_start(out=ag3_in[:, :], in_=hT_sb)

            # ---- AG3 + reload h_^T for all nodes
            ag(ag3_in, ag3_out)

            # x_ head runs while AG3 is in flight
            xo_sb = psm.tile([P, MT, IN], f32, name="xo_sb")
            for m in range(MT):
                ps = prot.tile([P, IN], f32, name="xo_ps", tag="rot")
                nc.tensor.matmul(
                    ps, a2_sb[:, ds(m * P, P)], dwx_sb, start=True, stop=True
                )
                nc.scalar.activation(
                    xo_sb[:, m, :], ps, Copy, scale=dv_sb[:, m : m + 1]
                )
            nc.scalar.dma_start(
                out=x_rows.rearrange("(m p) f -> p m f", p=P), in_=xo_sb
            )
            hTf_sb = ph.tile([IN, NCORES, NB], bf16, name="hTf_sb", tag="hbuf")
            ag3v = ag3_out.rearrange("(r f) m -> f r m", f=IN)
            nc.sync.dma_start(out=hTf_sb[:, 0:1, :], in_=ag3v[:, 0:1, :])
            nc.sync.dma_start(
                out=hTf_sb[:, 1:NCORES, :], in_=ag3v[:, 1:NCORES, :]
            )

            # ---- stage K: s_ rows = h_rows @ h_full^T (32 MB f32 out)
            dma_engines = [nc.sync, nc.scalar, nc.gpsimd]
            for m in range(MT):
                lhsT = hT_sb[:, ds(m * P, P)]
                for g in range(8):  # groups of 2 n-tiles -> one 512 KB DMA
                    ev = psm.tile([P, 1024], f32, name="s_ev", tag="sev", bufs=6)
                    for i in range(2):
                        n = g * 2 + i
                        ps = pacc.tile(
                            [P, 512], f32, name="s_ps",
                            tag=["accA", "accB", "accC", "accD"][n % 4],
                        )
                        nc.tensor.matmul(
                            ps,
                            lhsT,
                            hTf_sb[:, n // 2, ds((n % 2) * 512, 512)],
                            start=True,
                            stop=True,
                        )
                        if i % 2 == 0:
                            nc.scalar.copy(out=ev[:, ds(i * 512, 512)], in_=ps)
                        else:
                            nc.vector.tensor_copy(
                                out=ev[:, ds(i * 512, 512)], in_=ps
                            )
                    dma_engines[g % 3].dma_start(
                        out=s_rows[ds(m * P, P), ds(g * 1024, 1024)], in_=ev
                    )

    nc.finalize()
    return nc


def _get_program():
    global _PROG
    if _PROG is None:
        _PROG = _build_program()
    return _PROG


def kernel(**inputs) -> tuple:
    global LAST_EXEC_NS
    import ml_dtypes

    from concourse.bass_utils import run_bass_kernel_spmd

    bf = ml_dtypes.bfloat16

    x = np.asarray(inputs["x"], dtype=np.float32)
    src = np.asarray(inputs["src"]).astype(np.int64)
    dst = np.asarray(inputs["dst"]).astype(np.int64)

    # ---- host-side: bake the normalized adjacency (transposed), per hint:
    # edge partitioning by dst == column shards of A_hat^T.
    deg = 1.0 + np.bincount(dst, minlength=N).astype(np.float64)
    dinv = (1.0 / np.sqrt(deg)).astype(np.float32)
    # A'^T = (A + I)^T as exact small-integer edge counts (fp8-representable)
    flat = np.bincount(src * N + dst, minlength=N * N)
    assert flat.max() <= 16, "edge multiplicity too large for exact fp8"
    AT = flat.astype(np.float32).reshape(N, N)
    AT[np.arange(N), np.arange(N)] += 1.0

    w1 = np.concatenate(
        [inputs["enc_a_W1"], inputs["enc_s_W1"], inputs["enc_t_W1"]], axis=1
    )
    w2 = np.concatenate(
        [inputs["enc_a_W2"], inputs["enc_s_W2"], inputs["enc_t_W2"]], axis=1
    )
    def to_sbuf_layout(mat):
        # [N, F] node-major -> [128, KC*F]: row p holds chunks k of node k*128+p
        f = mat.shape[1]
        return np.ascontiguousarray(
            mat.reshape(KC, P, f).transpose(1, 0, 2).reshape(P, KC * f)
        )

    f8 = ml_dtypes.float8_e4m3

    common = {
        "xb": to_sbuf_layout((dinv[:, None] * x).astype(bf)),
        "w1": np.ascontiguousarray(np.asarray(w1, np.float32).astype(bf)),
        "w2": np.ascontiguousarray(np.asarray(w2, np.float32).astype(bf)),
        "aw": np.ascontiguousarray(np.asarray(inputs["att_W"], np.float32).astype(bf)),
        "dwx": np.ascontiguousarray(
            np.asarray(inputs["dec_x_W"], np.float32).astype(bf)
        ),
        "dws": np.ascontiguousarray(
            np.asarray(inputs["dec_s_W"], np.float32).astype(bf)
        ),
    }
    in_maps = []
    for r in range(NCORES):
        m = dict(common)
        m["at"] = to_sbuf_layout(AT[:, r * NB : (r + 1) * NB].astype(f8))
        dblk = dinv[r * NB : (r + 1) * NB]
        m["dv"] = np.ascontiguousarray(dblk.reshape(MT, P).T)
        m["dv2"] = np.ascontiguousarray((dblk * dblk).reshape(MT, P).T)
        m["dvr"] = np.ascontiguousarray(np.broadcast_to(dblk[None, :], (IN, NB)))
        in_maps.append(m)

    nc = _get_program()
    kwargs = {}
    if TRACE:
        kwargs = dict(trace=True, trace_cores=list(range(NCORES)))
    res = run_bass_kernel_spmd(nc, in_maps, core_ids=list(range(NCORES)), **kwargs)
    LAST_EXEC_NS = res.exec_time_ns
    results = res.results

    s_ = np.concatenate([results[r]["s_rows"] for r in range(NCORES)], axis=0)
    x_ = np.concatenate([results[r]["x_rows"] for r in range(NCORES)], axis=0)
    att = np.concatenate(
        [results[r]["att_rows"] for r in range(NCORES)], axis=0
    ).reshape(N, HID, 3)
    return (
        np.asarray(x_, np.float32),
        np.asarray(s_, np.float32),
        np.asarray(att, np.float32),
    )
